# revision 1
# baseline (speedup 1.0000x reference)
"""DeepSeek-V2 decode layer on 8 TRN2 NeuronCores (Bass/Tile SPMD kernel).

Sharding (per core c of 8):
  - QKV proj: row-parallel (528 of 4224 used rows each), then AllToAll to
    redistribute q^T from row-sharded to batch-sharded layout.
  - Attention: data-parallel over batch (8 sequences per core); kT cache fed
    host-transposed (d-major) so scores matmul needs no on-device transpose.
  - Wo: output-column-parallel (512 cols each) after AllGather of ctx.
  - post-attn hidden: AllGather of per-core column slices -> replicated.
  - MoE: expert-parallel (expert c on core c), dense compute for all 64
    tokens, combine weights applied per-core, AllReduce sum at the end.
  - rmsnorm weights (the x64 normalization factor and attention scale) are
    folded into the weight matrices host-side; the 1/sqrt(ms) per-token
    scale is applied at the qkv/gate/w1-input stage; all weights are fed
    pre-transposed so the contraction dim lands on SBUF partitions.
Matmuls run as float32r (full PE rate at moving-dim>=256, ~fp22 precision).
DMA: weight streams ride the SP HWDGE queue, traced in consumption order so
they prefetch from t=0; latency-critical per-phase traffic rides the ACT
HWDGE queue so it is never stuck behind bulk weight transfers.
"""

import os
import sys

import numpy as np

for _p in ("/opt/trn_rl_repo", "/root/.axon_site/_ro/trn_rl_repo", "/root/.axon_site"):
    if _p not in sys.path and os.path.isdir(_p):
        sys.path.append(_p)

def _ensure_ntff_hook():
    """This image's antenv lacks axon_hooks; shim it so BASS_TRACE works."""
    import types

    try:
        import antenv.axon_hooks  # noqa: F401
        return
    except ImportError:
        pass
    import antenv

    mod = types.ModuleType("antenv.axon_hooks")
    _state = {"h": None}
    mod.set_axon_ntff_profile_hook = lambda h: _state.__setitem__("h", h)
    mod.get_axon_ntff_profile_hook = lambda: _state["h"]
    sys.modules["antenv.axon_hooks"] = mod
    antenv.axon_hooks = mod
    try:
        sys.path.insert(0, "/root/.axon_site/trn_agent_boot")
        import trn_boot

        so_path = "/opt/axon/libaxon_pjrt.so"
        if os.path.exists(so_path):
            mod.set_axon_ntff_profile_hook(
                trn_boot._ntff_profile_via_ctypes(so_path))
    except Exception as e:  # tracing degrades; compile+run still work
        print(f"ntff hook install failed: {e}")


_ensure_ntff_hook()

import concourse.bacc as bacc
import concourse.bass as bass
import concourse.mybir as mybir
import concourse.tile as tile
from concourse.bass_utils import run_bass_kernel_spmd
from concourse.masks import make_identity
from contextlib import ExitStack

F32 = mybir.dt.float32
F32R = mybir.dt.float32r
AF = mybir.ActivationFunctionType
ALU = mybir.AluOpType

B, HID, S, NH, HD = 64, 4096, 4096, 32, 128
QROWS = NH * HD + HD          # 4224 used rows of Wqkv (q + current-k)
RPC = QROWS // 8              # 528 qkv rows per core
NB = B // 8                   # 8 batches per core
MI, TWO_MI = 1408, 2816
NC_ = 8
EPS = 1e-6

LAST_RESULT = None            # BassKernelResults of the most recent run


def _r(ap):
    return ap.bitcast(F32R)


_rr = _r  # producer-side relabel: walrus requires fp32r-typed producers


def _build_program():
    nc = bacc.Bacc(None, target_bir_lowering=False, num_devices=NC_)

    hid_i = nc.dram_tensor("hid", [B, HID], F32, kind="ExternalInput")
    hidT_i = nc.dram_tensor("hidT", [128, 32 * 64], F32, kind="ExternalInput")
    hidc_i = nc.dram_tensor("hidcols", [B, 512], F32, kind="ExternalInput")
    wqkvT_i = nc.dram_tensor("wqkvT", [HID, RPC], F32, kind="ExternalInput")
    woT_i = nc.dram_tensor("woT", [HID, 512], F32, kind="ExternalInput")
    gateT_i = nc.dram_tensor("gateT", [HID, 8], F32, kind="ExternalInput")
    w1T_i = nc.dram_tensor("w1T", [HID, TWO_MI], F32, kind="ExternalInput")
    w2T_i = nc.dram_tensor("w2T", [MI, HID], F32, kind="ExternalInput")
    kT_i = nc.dram_tensor("kT", [NB, HD, S], F32, kind="ExternalInput")
    v_i = nc.dram_tensor("v", [NB, 128, 32 * HD], F32, kind="ExternalInput")
    seqm1_i = nc.dram_tensor("seqm1", [1, NB], F32, kind="ExternalInput")
    sel_i = nc.dram_tensor("sel", [1, 8], F32, kind="ExternalInput")
    iota_i = nc.dram_tensor("iota2d", [128, 32], F32, kind="ExternalInput")
    ones_i = nc.dram_tensor("ones", [128, 1], F32, kind="ExternalInput")
    out_o = nc.dram_tensor("out", [B, HID], F32, kind="ExternalOutput")

    rg = [list(range(NC_))]

    with tile.TileContext(nc) as tc, ExitStack() as top:
        dramp = top.enter_context(tc.tile_pool(name="dram", bufs=1, space="DRAM"))
        a2a_in = dramp.tile([QROWS, NB], F32)
        a2a_out = dramp.tile([QROWS, NB], F32)
        ctx_b = dramp.tile([NB, HID], F32)
        agc = dramp.tile([B, HID], F32, addr_space="Shared")
        hsl_b = dramp.tile([B, 512], F32)
        agh = dramp.tile([B * 8, 512], F32, addr_space="Shared")
        moe_b = dramp.tile([B, HID], F32)
        ar_o = dramp.tile([B, HID], F32, addr_space="Shared")

        const = top.enter_context(tc.tile_pool(name="const", bufs=1))
        ident64 = const.tile([64, 64], F32)
        make_identity(nc, ident64)
        ident128 = const.tile([128, 128], F32)
        make_identity(nc, ident128)
        ones_col = const.tile([128, 1], F32)
        nc.scalar.dma_start(_rr(ones_col[:]), _rr(ones_i[:]))
        zero_col = const.tile([128, 1], F32)
        nc.gpsimd.memset(zero_col[:], 0.0)
        eps_col = const.tile([128, 1], F32)
        nc.gpsimd.memset(eps_col[:], float(HID) * EPS)
        nc.const_aps.aps[(F32, 0.0)] = zero_col[:]
        nc.const_aps.aps[(F32, float(HID) * EPS)] = eps_col[:]
        iota_sb = const.tile([128, 32], F32)
        nc.scalar.dma_start(iota_sb[:], iota_i[:])
        sel_bc = const.tile([64, 8], F32)
        nc.scalar.dma_start(sel_bc[:], sel_i.ap().to_broadcast((64, 8)))

        # long-lived activations
        acts = top.enter_context(tc.tile_pool(name="acts", bufs=1))
        scratch = acts.tile([B, HID], F32)       # square scratch / moe / final
        hid_full = acts.tile([B, HID], F32)      # post-attn residual hidden
        xT = acts.tile([128, 32 * 64], F32)
        midT = acts.tile([128, 11 * 64], F32)
        small = top.enter_context(tc.tile_pool(name="small", bufs=1))

        # ---------------- Phase A: norm1 -> qkv -> A2A ----------------
        with ExitStack() as pa:
            sA = pa.enter_context(tc.tile_pool(name="sA", bufs=1))
            ptA = pa.enter_context(tc.tile_pool(name="ptA", bufs=2, space="PSUM"))
            qkvps = pa.enter_context(tc.tile_pool(name="qkvps", bufs=1, space="PSUM"))
            wqp = pa.enter_context(tc.tile_pool(name="wqp", bufs=3))

            hT = sA.tile([128, 32 * 64], F32)
            nc.scalar.dma_start(_rr(hT[:]), _rr(hidT_i[:]))
            h_sb = sA.tile([B, HID], F32)
            nc.scalar.dma_start(h_sb[:], hid_i[:])
            ssq = small.tile([64, 1], F32, name="ssq")
            nc.scalar.activation(scratch[:], h_sb[:], AF.Square, accum_out=ssq[:])
            rs_col = small.tile([64, 1], F32, name="rs_col")
            nc.scalar.activation(rs_col[:], ssq[:], AF.Sqrt, bias=float(HID) * EPS)
            nc.vector.reciprocal(rs_col[:], rs_col[:])

            q1 = qkvps.tile([64, 512], F32, name="q1")
            q2 = qkvps.tile([64, 16], F32, name="q2")
            for k in range(32):
                wq = wqp.tile([128, RPC], F32, name="wq", tag="wq")
                nc.sync.dma_start(_rr(wq[:]), _rr(wqkvT_i[k * 128:(k + 1) * 128, :]))
                nc.tensor.matmul(q1[:], _r(hT[:, k * 64:(k + 1) * 64]),
                                 _r(wq[:, :512]), start=(k == 0), stop=(k == 31))
                nc.tensor.matmul(q2[:], _r(hT[:, k * 64:(k + 1) * 64]),
                                 _r(wq[:, 512:RPC]), start=(k == 0), stop=(k == 31))
            qkv_sb = sA.tile([64, RPC], F32)
            nc.vector.tensor_scalar_mul(qkv_sb[:, :512], q1[:], rs_col[:])
            nc.vector.tensor_scalar_mul(qkv_sb[:, 512:RPC], q2[:], rs_col[:])

            # transpose (64, 528) -> chunks of (128, 64), scatter into A2A input
            # a2a_in flat block s (rows 528s..528s+528) = qkv^T[:, 8s:8s+8]
            a2a_view = a2a_in.rearrange("(s q) j -> q s j", s=8)  # (528, 8, 8)
            for jt in range(5):
                rows = 128 if jt < 4 else 16
                pt = ptA.tile([128, 64], F32, name="ptA_q", tag="ptA_t")
                nc.tensor.transpose(pt[:rows, :],
                                    qkv_sb[:, jt * 128: jt * 128 + rows], ident64[:])
                qs = sA.tile([128, 64], F32, name="qs", tag="qs", bufs=2)
                nc.vector.tensor_copy(qs[:rows, :], pt[:rows, :])
                src = qs[:rows, :].rearrange("p (s j) -> p s j", s=8)
                nc.scalar.dma_start(a2a_view[jt * 128: jt * 128 + rows], src)

            nc.gpsimd.collective_compute(
                "AllToAll", ALU.bypass, replica_groups=rg,
                ins=[a2a_in.opt()], outs=[a2a_out.opt()],
            )

        # ------- weight streams (SP queue, traced in consumption order) -------
        wop = top.enter_context(tc.tile_pool(name="wop", bufs=3))
        gwp = top.enter_context(tc.tile_pool(name="gwp", bufs=2))
        w1p = top.enter_context(tc.tile_pool(name="w1p", bufs=4))
        wo_tiles, gw_tiles, w1_tiles = [], [], []
        for k in range(32):
            wo_t = wop.tile([128, 512], F32, name="wo_t", tag="wo")
            nc.sync.dma_start(_rr(wo_t[:]), _rr(woT_i[k * 128:(k + 1) * 128, :]))
            wo_tiles.append(wo_t)
        for k in range(32):
            gw = gwp.tile([128, 8], F32, name="gw", tag="gw")
            nc.sync.dma_start(_rr(gw[:]), _rr(gateT_i[k * 128:(k + 1) * 128, :]))
            gw_tiles.append(gw)
        for k in range(32):
            w1t = w1p.tile([128, TWO_MI], F32, name="w1t", tag="w1")
            for hh in range(2):
                fs = slice(hh * MI, (hh + 1) * MI)
                nc.sync.dma_start(_rr(w1t[:, fs]),
                                  _rr(w1T_i[k * 128:(k + 1) * 128, fs]))
            w1_tiles.append(w1t)

        # ---------------- Phase B: attention (8 local batches) ----------------
        with ExitStack() as pb:
            sB = pb.enter_context(tc.tile_pool(name="sB", bufs=1))
            kvp = pb.enter_context(tc.tile_pool(name="kvp", bufs=2))
            ppp = pb.enter_context(tc.tile_pool(name="ppp", bufs=2))
            smb = pb.enter_context(tc.tile_pool(name="smb", bufs=2))
            drb = pb.enter_context(tc.tile_pool(name="drb", bufs=2, space="DRAM"))
            scps = pb.enter_context(tc.tile_pool(name="scps", bufs=2, space="PSUM"))
            dps = pb.enter_context(tc.tile_pool(name="dps", bufs=2, space="PSUM"))
            ctxps = pb.enter_context(tc.tile_pool(name="ctxps", bufs=2, space="PSUM"))
            ctnps = pb.enter_context(tc.tile_pool(name="ctnps", bufs=2, space="PSUM"))

            # q^T for all local batches: (128, kk=33, j=8); row kk*128+p of A2A out
            qT = sB.tile([128, 33, NB], F32)
            nc.scalar.dma_start(_rr(qT[:]),
                                _rr(a2a_out.rearrange("(kk p) j -> p kk j", p=128)))

            for j in range(NB):
                kT_sb = kvp.tile([128, S], F32, name="kT_sb", tag="kT")
                for hh in range(4):
                    sl = slice(hh * (S // 4), (hh + 1) * (S // 4))
                    nc.scalar.dma_start(_rr(kT_sb[:, sl]), _rr(kT_i[j][:, sl]))
                v_sb = kvp.tile([128, 32, HD], F32, name="v_sb", tag="v")
                vv = v_i[j].rearrange("p (c d) -> p c d", d=HD)
                for hh in range(4):
                    sl = slice(hh * 8, (hh + 1) * 8)
                    nc.scalar.dma_start(_rr(v_sb[:, sl, :]), _rr(vv[:, sl, :]))
                sv_col = smb.tile([128, 1], F32, name="sv_col", tag="sv")
                nc.scalar.dma_start(sv_col[:],
                                    seqm1_i[0:1, j:j + 1].to_broadcast((128, 1)))
                vcur = smb.tile([1, HD], F32, name="vcur", tag="vcur")
                nc.scalar.dma_start(
                    _rr(vcur[:]),
                    _rr(a2a_out.rearrange("q j -> j q")[j:j + 1, 4096:4224]))

                qT_b = qT[:, 0:32, j]          # (128, 32) strided: q^T for batch
                s0 = scps.tile([128, 512], F32, name="s0", tag="sc")
                s1 = scps.tile([128, 512], F32, name="s1", tag="sc")
                for c in range(32):
                    dst = (s0 if c < 16 else s1)[:, (c % 16) * 32:(c % 16) * 32 + 32]
                    nc.tensor.matmul(dst, _r(kT_sb[:, c * 128:(c + 1) * 128]),
                                     _r(qT_b), start=True, stop=True)
                cur = dps.tile([1, 32], F32, name="cur", tag="dsm")
                nc.tensor.matmul(cur[:], _r(qT[:, 32, j:j + 1]), _r(qT_b),
                                 start=True, stop=True)

                pp = ppp.tile([128, 1024], F32, name="pp", tag="pp")
                nc.scalar.activation(_rr(pp[:, :512]), s0[:], AF.Exp)
                nc.scalar.activation(_rr(pp[:, 512:]), s1[:], AF.Exp)
                pcur = smb.tile([1, 32], F32, name="pcur", tag="pcur")
                nc.scalar.activation(_rr(pcur[:]), cur[:], AF.Exp)

                m0 = smb.tile([128, 16], F32, name="m0", tag="m0")
                m1 = smb.tile([128, 16], F32, name="m1", tag="m1")
                nc.vector.tensor_scalar(m0[:], iota_sb[:, 0:16], sv_col[:], None,
                                        op0=ALU.is_lt)
                nc.vector.tensor_scalar(m1[:], iota_sb[:, 16:32], sv_col[:], None,
                                        op0=ALU.is_lt)
                pp3 = pp[:].rearrange("p (c h) -> p c h", h=32)
                nc.vector.tensor_tensor(
                    _rr(pp3[:, 0:16]), pp3[:, 0:16],
                    m0[:, :, None].to_broadcast((128, 16, 32)), op=ALU.mult)
                nc.vector.tensor_tensor(
                    _rr(pp3[:, 16:32]), pp3[:, 16:32],
                    m1[:, :, None].to_broadcast((128, 16, 32)), op=ALU.mult)

                dsum = dps.tile([1, 512], F32, name="dsum", tag="dsm")
                nc.tensor.matmul(dsum[:], _r(ones_col[:]), _r(pp[:, :512]),
                                 start=True, stop=False)
                nc.tensor.matmul(dsum[:], _r(ones_col[:]), _r(pp[:, 512:]),
                                 start=False, stop=True)
                den = smb.tile([1, 32], F32, name="den", tag="den")
                nc.vector.reduce_sum(den[:],
                                     dsum[:].rearrange("p (c h) -> p h c", h=32),
                                     axis=mybir.AxisListType.X)
                nc.vector.tensor_tensor(den[:], den[:], pcur[:], op=ALU.add)
                rden = smb.tile([1, 32], F32, name="rden", tag="rden")
                nc.vector.reciprocal(rden[:], den[:])
                rd_d = drb.tile([1, 32], F32, name="rd_d", tag="rd")
                nc.scalar.dma_start(rd_d[:], rden[:])
                rden_bc = smb.tile([128, 32], F32, name="rden_bc", tag="rdbc")
                nc.scalar.dma_start(rden_bc[:], rd_d.to_broadcast((128, 32)))

                ctx = ctxps.tile([128, 32], F32, name="ctx", tag="ctx")
                for c in range(32):
                    nc.tensor.matmul(ctx[:], _r(v_sb[:, c, :]),
                                     _r(pp[:, c * 32:(c + 1) * 32]),
                                     start=(c == 0), stop=False)
                nc.tensor.matmul(ctx[:], _r(vcur[:]), _r(pcur[:]),
                                 start=False, stop=True)
                ctxT_sb = smb.tile([128, 32], F32, name="ctxT_sb", tag="ctxs")
                nc.vector.tensor_tensor(ctxT_sb[:], ctx[:], rden_bc[:], op=ALU.mult)
                ctn = ctnps.tile([32, 128], F32, name="ctn", tag="ctn")
                nc.tensor.transpose(ctn[:], ctxT_sb[:], ident128[:])
                ctn_sb = smb.tile([32, 128], F32, name="ctn_sb", tag="ctns")
                nc.vector.tensor_copy(ctn_sb[:], ctn[:])
                nc.scalar.dma_start(
                    ctx_b[j:j + 1, :].rearrange("o (h d) -> h (o d)", d=HD),
                    ctn_sb[:])

        # ---------------- Phase C: AG ctx -> Wo -> residual -> AG hidden ------
        with ExitStack() as pc:
            sC = pc.enter_context(tc.tile_pool(name="sC", bufs=1))
            wops = pc.enter_context(tc.tile_pool(name="wops", bufs=1, space="PSUM"))
            ptC = pc.enter_context(tc.tile_pool(name="ptC", bufs=2, space="PSUM"))

            nc.gpsimd.collective_compute(
                "AllGather", ALU.bypass, replica_groups=rg,
                ins=[ctx_b.opt()], outs=[agc.opt()],
            )
            ctx_all = sC.tile([B, HID], F32)
            nc.scalar.dma_start(ctx_all[:], agc[:])
            ctxA = sC.tile([128, 32 * 64], F32)
            for k in range(32):
                pt = ptC.tile([128, 64], F32, name="ptC_t", tag="ptC_t")
                nc.tensor.transpose(pt[:], ctx_all[:, k * 128:(k + 1) * 128],
                                    ident64[:])
                nc.vector.tensor_copy(_rr(ctxA[:, k * 64:(k + 1) * 64]), _rr(pt[:]))

            wo_ps = wops.tile([64, 512], F32)
            for k in range(32):
                nc.tensor.matmul(wo_ps[:], _r(ctxA[:, k * 64:(k + 1) * 64]),
                                 _r(wo_tiles[k][:]),
                                 start=(k == 0), stop=(k == 31))
            hidc = sC.tile([64, 512], F32)
            nc.scalar.dma_start(hidc[:], hidc_i[:])
            hsl = sC.tile([64, 512], F32)
            nc.vector.tensor_tensor(hsl[:], wo_ps[:], hidc[:], op=ALU.add)
            nc.scalar.dma_start(hsl_b[:], hsl[:])
            nc.gpsimd.collective_compute(
                "AllGather", ALU.bypass, replica_groups=rg,
                ins=[hsl_b.opt()], outs=[agh.opt()],
            )
            nc.scalar.dma_start(hid_full[:].rearrange("b (r o) -> b r o", r=8),
                                agh.rearrange("(r b) o -> b r o", b=64))

        # ---------------- Phase D: norm2 -> x^T -> gate -> top2 ----------------
        wsel_col = small.tile([64, 1], F32, name="wsel_col")
        with ExitStack() as pd:
            sD = pd.enter_context(tc.tile_pool(name="sD", bufs=1))
            ptD = pd.enter_context(tc.tile_pool(name="ptD", bufs=2, space="PSUM"))
            gps = pd.enter_context(tc.tile_pool(name="gps", bufs=1, space="PSUM"))

            ssq2 = small.tile([64, 1], F32, name="ssq2")
            nc.scalar.activation(scratch[:], hid_full[:], AF.Square,
                                 accum_out=ssq2[:])
            rs2 = small.tile([64, 1], F32, name="rs2")
            nc.scalar.activation(rs2[:], ssq2[:], AF.Sqrt, bias=float(HID) * EPS)
            nc.vector.reciprocal(rs2[:], rs2[:])
            x_sb = sD.tile([B, HID], F32)
            nc.vector.tensor_scalar_mul(x_sb[:], hid_full[:], rs2[:])

            for k in range(32):
                pt = ptD.tile([128, 64], F32, name="ptD_t", tag="ptD_t")
                nc.tensor.transpose(pt[:], x_sb[:, k * 128:(k + 1) * 128], ident64[:])
                nc.vector.tensor_copy(_rr(xT[:, k * 64:(k + 1) * 64]), _rr(pt[:]))

            g_ps = gps.tile([64, 8], F32)
            for k in range(32):
                nc.tensor.matmul(g_ps[:], _r(xT[:, k * 64:(k + 1) * 64]),
                                 _r(gw_tiles[k][:]),
                                 start=(k == 0), stop=(k == 31))
            pg = sD.tile([64, 8], F32)
            nc.scalar.activation(pg[:], g_ps[:], AF.Exp)
            m1c = sD.tile([64, 1], F32)
            nc.vector.reduce_max(m1c[:], pg[:], axis=mybir.AxisListType.X)
            eq1 = sD.tile([64, 8], F32)
            nc.vector.tensor_scalar(eq1[:], pg[:], m1c[:], None, op0=ALU.is_ge)
            t1 = sD.tile([64, 8], F32)
            nc.vector.tensor_tensor(t1[:], pg[:], eq1[:], op=ALU.mult)
            nc.vector.tensor_tensor(t1[:], pg[:], t1[:], op=ALU.subtract)
            m2c = sD.tile([64, 1], F32)
            nc.vector.reduce_max(m2c[:], t1[:], axis=mybir.AxisListType.X)
            keep = sD.tile([64, 8], F32)
            nc.vector.tensor_scalar(keep[:], pg[:], m2c[:], None, op0=ALU.is_ge)
            wsum = sD.tile([64, 1], F32)
            nc.vector.tensor_tensor(wsum[:], m1c[:], m2c[:], op=ALU.add)
            nc.vector.reciprocal(wsum[:], wsum[:])
            wts = sD.tile([64, 8], F32)
            nc.vector.tensor_tensor(wts[:], pg[:], keep[:], op=ALU.mult)
            nc.vector.tensor_scalar_mul(wts[:], wts[:], wsum[:])
            nc.vector.tensor_tensor(wts[:], wts[:], sel_bc[:], op=ALU.mult)
            nc.vector.reduce_sum(wsel_col[:], wts[:], axis=mybir.AxisListType.X)

        # ---------------- Phase E: MoE expert FFN + AllReduce ----------------
        with ExitStack() as pe1:
            gups = pe1.enter_context(tc.tile_pool(name="gups", bufs=1, space="PSUM"))
            ptE = pe1.enter_context(tc.tile_pool(name="ptE", bufs=2, space="PSUM"))
            sE = pe1.enter_context(tc.tile_pool(name="sE", bufs=1))

            gu = gups.tile([64, TWO_MI], F32)
            slices = [(o * 512, min(512, TWO_MI - o * 512)) for o in range(6)]
            for k in range(32):
                w1t = w1_tiles[k]
                for (off, w) in slices:
                    nc.tensor.matmul(gu[:, off:off + w],
                                     _r(xT[:, k * 64:(k + 1) * 64]),
                                     _r(w1t[:, off:off + w]),
                                     start=(k == 0), stop=(k == 31))
            sg = sE.tile([64, MI], F32)
            nc.scalar.activation(sg[:], gu[:, :MI], AF.Silu)
            mid = sE.tile([64, MI], F32)
            nc.vector.tensor_tensor(mid[:], sg[:], gu[:, MI:], op=ALU.mult)

            for mk in range(11):
                pt = ptE.tile([128, 64], F32, name="ptE_t", tag="ptE_t")
                nc.tensor.transpose(pt[:], mid[:, mk * 128:(mk + 1) * 128],
                                    ident64[:])
                nc.vector.tensor_copy(_rr(midT[:, mk * 64:(mk + 1) * 64]), _rr(pt[:]))

        with ExitStack() as pe2:
            w2p = pe2.enter_context(tc.tile_pool(name="w2p", bufs=3))
            mops = pe2.enter_context(tc.tile_pool(name="mops", bufs=2, space="PSUM"))
            sF = pe2.enter_context(tc.tile_pool(name="sF", bufs=1))
            w2_tiles = []
            for mk in range(11):
                w2t = w2p.tile([128, HID], F32, name="w2t", tag="w2")
                for hh in range(2):
                    fs = slice(hh * 2048, (hh + 1) * 2048)
                    nc.sync.dma_start(_rr(w2t[:, fs]),
                                      _rr(w2T_i[mk * 128:(mk + 1) * 128, fs]))
                w2_tiles.append(w2t)
            mo0 = mops.tile([64, 2048], F32, name="mo0", tag="mo")
            mo1 = mops.tile([64, 2048], F32, name="mo1", tag="mo")
            for mk in range(11):
                w2t = w2_tiles[mk]
                for oh, mo in ((0, mo0), (1, mo1)):
                    for oc in range(4):
                        off = oh * 2048 + oc * 512
                        nc.tensor.matmul(mo[:, oc * 512:(oc + 1) * 512],
                                         _r(midT[:, mk * 64:(mk + 1) * 64]),
                                         _r(w2t[:, off:off + 512]),
                                         start=(mk == 0), stop=(mk == 10))
            nc.vector.tensor_scalar_mul(scratch[:, :2048], mo0[:], wsel_col[:])
            nc.vector.tensor_scalar_mul(scratch[:, 2048:], mo1[:], wsel_col[:])

            nc.scalar.dma_start(moe_b[:], scratch[:])
            nc.gpsimd.collective_compute(
                "AllReduce", ALU.add, replica_groups=rg,
                ins=[moe_b.opt()], outs=[ar_o.opt()],
            )
            ar_sb = sF.tile([B, HID], F32)
            nc.scalar.dma_start(ar_sb[:], ar_o[:])
            nc.vector.tensor_tensor(scratch[:], ar_sb[:], hid_full[:], op=ALU.add)
            nc.scalar.dma_start(out_o[:], scratch[:])

    nc.compile()
    return nc


_NC_CACHE = None


def _get_program():
    global _NC_CACHE
    if _NC_CACHE is None:
        _NC_CACHE = _build_program()
    return _NC_CACHE


def kernel(hidden_states, positions, k_cache, v_cache, seq_lens,
           norm1_w, norm2_w, Wqkv, Wo, gate_w, w1, w2):
    global LAST_RESULT
    nc = _get_program()

    hs = np.asarray(hidden_states, np.float32).reshape(B, HID)
    scale = np.float32(HD) ** -0.5
    n1 = (np.asarray(norm1_w, np.float32) * 64.0)
    n2 = (np.asarray(norm2_w, np.float32) * 64.0)

    wq = np.asarray(Wqkv, np.float32)[:QROWS] * n1[None, :]
    wq[:NH * HD] *= scale
    gT = np.ascontiguousarray((np.asarray(gate_w, np.float32) * n2[None, :]).T)
    iota2d = (np.arange(128, dtype=np.float32)[:, None]
              + 128.0 * np.arange(32, dtype=np.float32)[None, :])
    seqm1 = (np.asarray(seq_lens, np.int32).astype(np.float32) - 1.0)
    # hidT[p, k*64+b] = hs[b, 128k+p]
    hidT = np.ascontiguousarray(
        hs.T.reshape(32, 128, 64).transpose(1, 0, 2).reshape(128, 32 * 64))

    in_maps = []
    for c in range(NC_):
        bs = slice(c * NB, (c + 1) * NB)
        sel = np.zeros((1, 8), np.float32)
        sel[0, c] = 1.0
        in_maps.append({
            "hid": hs,
            "hidT": hidT,
            "hidcols": np.ascontiguousarray(hs[:, c * 512:(c + 1) * 512]),
            "wqkvT": np.ascontiguousarray(wq[c * RPC:(c + 1) * RPC].T),
            "woT": np.ascontiguousarray(
                np.asarray(Wo, np.float32)[c * 512:(c + 1) * 512].T),
            "gateT": gT,
            "w1T": np.ascontiguousarray((np.asarray(w1, np.float32)[c]
                                         * n2[None, :]).T),
            "w2T": np.ascontiguousarray(np.asarray(w2, np.float32)[c].T),
            "kT": np.ascontiguousarray(
                np.asarray(k_cache, np.float32)[bs].transpose(0, 2, 1)),
            "v": np.ascontiguousarray(np.asarray(v_cache, np.float32)[bs]
                                      .reshape(NB, 32, 128, HD)
                                      .transpose(0, 2, 1, 3)
                                      .reshape(NB, 128, 32 * HD)),
            "seqm1": np.ascontiguousarray(seqm1[bs].reshape(1, NB)),
            "sel": sel,
            "iota2d": iota2d,
            "ones": np.ones((128, 1), np.float32),
        })

    LAST_RESULT = run_bass_kernel_spmd(nc, in_maps, core_ids=list(range(NC_)))
    return LAST_RESULT.results[0]["out"].reshape(B, 1, HID).astype(np.float32)



# revision 16
# speedup vs baseline: 1.8350x; 1.8350x over previous
"""DeepSeek-V2 decode layer on 8 TRN2 NeuronCores (Bass/Tile SPMD kernel), v2.

Sharding (per core c of 8):
  - QKV proj row-parallel (528 rows/core) in e3m4, AllToAll (bf16) to
    batch-sharded q^T layout.
  - Attention data-parallel over batch: batches sorted by seq_len and dealt
    round-robin so slot j has a uniform compile-time trip count
    tc[j] = ceil(max_slot_seq/128); KV cache streamed as e3m4 (x2 scale).
  - ctx computed directly in [d, h] layout (v-stationary matmuls) -> no
    transposes before Wo; AllGather (bf16) -> Wo col-parallel (e3m4).
  - hidden slice kept fp32 locally; transposed bf16 slices AllGathered for
    the MoE x^T path; fp32 slices AllGathered (overlapped with MoE compute)
    for the exact final residual.
  - MoE expert-parallel, dense over 64 tokens, w1/w2 in e3m4; combine weights
    carry the fp8 dequant scales; final AllReduce fp32 in 2 column chunks.
  - matmuls are column-pair tiled (tile_position (0,0)/(0,64)) where the
    token dim M=64 would otherwise waste half the PE array.
Dequant-scale folding: kq^2 into the rsqrt(ms) bias/scale, HD^-0.5/kkv into
the softmax Exp scale, kvv into Wo host-side, 1/(k1*k2) into the expert
one-hot `sel`, 1/kwo into the Wo-psum copy scale.
"""

import os
import sys
import math

import numpy as np
import ml_dtypes

for _p in ("/opt/trn_rl_repo", "/root/.axon_site/_ro/trn_rl_repo", "/root/.axon_site"):
    if _p not in sys.path and os.path.isdir(_p):
        sys.path.append(_p)


def _ensure_ntff_hook():
    """This image's antenv lacks axon_hooks; shim it so BASS_TRACE works."""
    import types

    try:
        import antenv.axon_hooks  # noqa: F401
        return
    except ImportError:
        pass
    import antenv

    mod = types.ModuleType("antenv.axon_hooks")
    _state = {"h": None}
    mod.set_axon_ntff_profile_hook = lambda h: _state.__setitem__("h", h)
    mod.get_axon_ntff_profile_hook = lambda: _state["h"]
    sys.modules["antenv.axon_hooks"] = mod
    antenv.axon_hooks = mod
    try:
        sys.path.insert(0, "/root/.axon_site/trn_agent_boot")
        import trn_boot

        so_path = "/opt/axon/libaxon_pjrt.so"
        if os.path.exists(so_path):
            mod.set_axon_ntff_profile_hook(
                trn_boot._ntff_profile_via_ctypes(so_path))
    except Exception as e:  # tracing degrades; compile+run still work
        print(f"ntff hook install failed: {e}")


_ensure_ntff_hook()

import concourse.bacc as bacc
import concourse.bass as bass
import concourse.mybir as mybir
import concourse.tile as tile
from concourse.bass_utils import run_bass_kernel_spmd
from concourse.masks import make_identity
from contextlib import ExitStack

F32 = mybir.dt.float32
BF16 = mybir.dt.bfloat16
E3 = mybir.dt.float8e3
AF = mybir.ActivationFunctionType
ALU = mybir.AluOpType

B, HID, S, NH, HD = 64, 4096, 4096, 32, 128
QROWS = NH * HD + HD          # 4224 used rows of Wqkv (q + current-k)
RPC = QROWS // 8              # 528 qkv rows per core
NB = B // 8                   # 8 batches (slots) per core
MI, TWO_MI = 1408, 2816
NC_ = 8
EPS = 1e-6
E3M4 = ml_dtypes.float8_e3m4
BF = ml_dtypes.bfloat16
KKV = 2.0                     # host scale on k cache
KVV = 2.0                     # host scale on v cache

LAST_RESULT = None            # BassKernelResults of the most recent run


def _pow2_scale(x, target=2.0):
    s = float(np.asarray(x, np.float32).std())
    if s <= 0:
        return 1.0
    return 2.0 ** round(math.log2(target / s))


def _e3(x):
    return np.clip(np.asarray(x, np.float32), -15.0, 15.0).astype(E3M4)


def _pack32(wT, nk, ncols):
    """[rows, K=nk*128] weight (row-major) -> [128, nk*ncols] chunk-packed:
    pack[p, k*ncols + r] = wT[r, k*128 + p]."""
    return np.ascontiguousarray(
        wT.T.reshape(nk, 128, ncols).transpose(1, 0, 2).reshape(128, nk * ncols))


def _build_program(tcs, kq, kwo, k1):
    nc = bacc.Bacc(None, target_bir_lowering=False, num_devices=NC_)

    hidT_i = nc.dram_tensor("hidT", [128, 32 * 64], BF16, kind="ExternalInput")
    h_i = nc.dram_tensor("hbf", [B, HID], BF16, kind="ExternalInput")
    hidc_i = nc.dram_tensor("hidcols", [B, 512], F32, kind="ExternalInput")
    wqkvT_i = nc.dram_tensor("wqkvT", [128, 32 * RPC], E3, kind="ExternalInput")
    woT_i = nc.dram_tensor("woT", [128, 32 * 512], E3, kind="ExternalInput")
    gateT_i = nc.dram_tensor("gateT", [128, 32 * 8], BF16, kind="ExternalInput")
    w1T_i = nc.dram_tensor("w1T", [128, 32 * TWO_MI], E3, kind="ExternalInput")
    w2T_i = nc.dram_tensor("w2T", [128, 11 * HID], E3, kind="ExternalInput")
    kT_i = nc.dram_tensor("kT", [NB, HD, S], E3, kind="ExternalInput")
    v_i = nc.dram_tensor("v", [NB, 128, 32 * HD], E3, kind="ExternalInput")
    seqm1_i = nc.dram_tensor("seqm1", [1, NB], F32, kind="ExternalInput")
    sel_i = nc.dram_tensor("sel", [1, 8], F32, kind="ExternalInput")
    iota_i = nc.dram_tensor("iota2d", [128, 32], F32, kind="ExternalInput")
    out_o = nc.dram_tensor("out", [B, HID], F32, kind="ExternalOutput")

    rg = [list(range(NC_))]
    exp_scale = float(HD) ** -0.5

    with tile.TileContext(nc) as tc, ExitStack() as top:
        dramp = top.enter_context(tc.tile_pool(name="dram", bufs=1, space="DRAM"))
        a2a_in = dramp.tile([QROWS, NB], BF16)
        a2a_out = dramp.tile([QROWS, NB], BF16)
        ctx_b = dramp.tile([NB, HD, 32], BF16)
        agc = dramp.tile([NC_, NB, HD, 32], BF16, addr_space="Shared")
        hslT_b = dramp.tile([4, 128, 64], BF16)
        aghT = dramp.tile([NC_, 4, 128, 64], BF16, addr_space="Shared")
        hsl_b = dramp.tile([B, 512], F32)
        agh = dramp.tile([NC_, B, 512], F32, addr_space="Shared")
        moe_b0 = dramp.tile([B, 2048], F32)
        moe_b1 = dramp.tile([B, 2048], F32)
        ar_o0 = dramp.tile([B, 2048], F32, addr_space="Shared")
        ar_o1 = dramp.tile([B, 2048], F32, addr_space="Shared")

        const = top.enter_context(tc.tile_pool(name="const", bufs=1))
        ident64b = const.tile([64, 64], BF16)
        make_identity(nc, ident64b)
        ones_bf = const.tile([128, 1], BF16)
        nc.gpsimd.memset(ones_bf[:], 1.0)
        ones_row = const.tile([1, 128], F32)
        nc.gpsimd.memset(ones_row[:], 1.0)
        zero_col = const.tile([128, 1], F32)
        nc.gpsimd.memset(zero_col[:], 0.0)
        nc.const_aps.aps[(F32, 0.0)] = zero_col[:]
        epsq_col = const.tile([128, 1], F32)
        nc.gpsimd.memset(epsq_col[:], float(HID) * EPS * kq * kq)
        nc.const_aps.aps[(F32, float(HID) * EPS * kq * kq)] = epsq_col[:]
        eps_col = const.tile([128, 1], F32)
        nc.gpsimd.memset(eps_col[:], float(HID) * EPS)
        nc.const_aps.aps[(F32, float(HID) * EPS)] = eps_col[:]
        iota_sb = const.tile([128, 32], F32)
        nc.scalar.dma_start(iota_sb[:], iota_i[:])
        sel_bc = const.tile([64, 8], F32)
        nc.scalar.dma_start(sel_bc[:], sel_i.ap().to_broadcast((64, 8)))

        small = top.enter_context(tc.tile_pool(name="small", bufs=1))
        # long-lived activations
        acts = top.enter_context(tc.tile_pool(name="acts", bufs=1))
        qT = acts.tile([128, 33, NB], BF16)
        ctxA = acts.tile([128, 64, 32], BF16)
        hidTf = acts.tile([128, 32, 64], BF16)
        xT = acts.tile([128, 32, 64], BF16)
        midT = acts.tile([128, 11 * 64], BF16)
        hsl_f = acts.tile([64, 512], F32)
        wsel_col = small.tile([64, 1], F32, name="wsel_col")

        # ---------------- Phase A: norm1 -> qkv -> A2A ----------------
        with ExitStack() as pa:
            sA = pa.enter_context(tc.tile_pool(name="sA", bufs=1))
            ptA = pa.enter_context(tc.tile_pool(name="ptA", bufs=2, space="PSUM"))
            qkvps = pa.enter_context(tc.tile_pool(name="qkvps", bufs=1, space="PSUM"))

            wq_sb = sA.tile([128, 32 * RPC], E3)
            nc.sync.dma_start(wq_sb[:], wqkvT_i[:])
            hT = sA.tile([128, 32 * 64], BF16)
            nc.scalar.dma_start(hT[:], hidT_i[:])
            h_sb = sA.tile([B, HID], BF16)
            nc.scalar.dma_start(h_sb[:], h_i[:])
            sq_scr = sA.tile([B, HID], BF16)
            ssq = small.tile([64, 1], F32, name="ssq")
            nc.scalar.activation(sq_scr[:], h_sb[:], AF.Square, accum_out=ssq[:])
            rs_col = small.tile([64, 1], F32, name="rs_col")
            # rs = 1/(kq*sqrt(ssq + HID*EPS)) : sqrt(ssq*kq^2 + HID*EPS*kq^2)
            nc.scalar.activation(rs_col[:], ssq[:], AF.Sqrt,
                                 bias=float(HID) * EPS * kq * kq, scale=kq * kq)
            nc.vector.reciprocal(rs_col[:], rs_col[:])

            q1 = qkvps.tile([128, 512], F32, name="q1")
            q2 = qkvps.tile([128, 16], F32, name="q2")
            for k in range(32):
                par = k % 2
                tp = (0, 64 * par)
                hk = hT[:, k * 64:(k + 1) * 64]
                wk = wq_sb[:, k * RPC:(k + 1) * RPC]
                nc.tensor.matmul(q1[64 * par:64 * par + 64, :], hk, wk[:, :512],
                                 start=(k < 2), stop=(k >= 30), tile_position=tp)
                nc.tensor.matmul(q2[64 * par:64 * par + 64, :], hk, wk[:, 512:RPC],
                                 start=(k < 2), stop=(k >= 30), tile_position=tp)
            qkv_hi = sA.tile([64, RPC], F32)
            nc.scalar.activation(qkv_hi[:, :512], q1[64:128, :], AF.Copy)
            nc.scalar.activation(qkv_hi[:, 512:RPC], q2[64:128, :], AF.Copy)
            qkv_f = sA.tile([64, RPC], F32)
            nc.vector.tensor_tensor(qkv_f[:, :512], q1[0:64, :],
                                    qkv_hi[:, :512], op=ALU.add)
            nc.vector.tensor_tensor(qkv_f[:, 512:RPC], q2[0:64, :],
                                    qkv_hi[:, 512:RPC], op=ALU.add)
            qkv_sb = sA.tile([64, RPC], BF16)
            nc.vector.tensor_scalar_mul(qkv_sb[:], qkv_f[:], rs_col[:])

            # transpose (64, 528) -> chunks of (128, 64), scatter into A2A input
            a2a_view = a2a_in.rearrange("(s q) j -> q s j", s=8)  # (528, 8, 8)
            for jt in range(5):
                rows = 128 if jt < 4 else 16
                pt = ptA.tile([128, 64], BF16, name="ptA_q", tag="ptA_t")
                nc.tensor.transpose(pt[:rows, :],
                                    qkv_sb[:, jt * 128: jt * 128 + rows],
                                    ident64b[:])
                qs = sA.tile([128, 64], BF16, name="qs", tag="qs", bufs=2)
                nc.vector.tensor_copy(qs[:rows, :], pt[:rows, :])
                src = qs[:rows, :].rearrange("p (s j) -> p s j", s=8)
                nc.scalar.dma_start(a2a_view[jt * 128: jt * 128 + rows], src)

            nc.gpsimd.collective_compute(
                "AllToAll", ALU.bypass, replica_groups=rg,
                ins=[a2a_in.opt()], outs=[a2a_out.opt()],
            )

        # ------- bulk weight streams (SP queue, consumption order) -------
        w1p = top.enter_context(tc.tile_pool(name="w1p", bufs=3))
        w2p = top.enter_context(tc.tile_pool(name="w2p", bufs=2))
        mid_scope = top.enter_context(ExitStack())
        wstr = mid_scope.enter_context(tc.tile_pool(name="wstr", bufs=1))
        wo_sb = wstr.tile([128, 32 * 512], E3)
        nc.sync.dma_start(wo_sb[:], woT_i[:])
        gate_sb = wstr.tile([128, 32 * 8], BF16)
        nc.sync.dma_start(gate_sb[:], gateT_i[:])
        w1_tiles = []
        for t in range(4):
            w1t = w1p.tile([128, 8 * TWO_MI], E3, name="w1t", tag="w1")
            nc.sync.dma_start(w1t[:], w1T_i[:, t * 8 * TWO_MI:(t + 1) * 8 * TWO_MI])
            w1_tiles.append(w1t)
        w2_tiles = []
        for t, nmk in ((0, 6), (1, 5)):
            w2t = w2p.tile([128, 6 * HID], E3, name="w2t", tag="w2")
            off = t * 6 * HID
            nc.sync.dma_start(w2t[:, :nmk * HID], w2T_i[:, off:off + nmk * HID])
            w2_tiles.append(w2t)

        # ---------------- Phase B: attention (8 slots) ----------------
        with ExitStack() as pb:
            kvp = pb.enter_context(tc.tile_pool(name="kvp", bufs=2))
            ppp = pb.enter_context(tc.tile_pool(name="ppp", bufs=2))
            smb = pb.enter_context(tc.tile_pool(name="smb", bufs=2))
            scps = pb.enter_context(tc.tile_pool(name="scps", bufs=2, space="PSUM"))
            dps = pb.enter_context(tc.tile_pool(name="dps", bufs=1, space="PSUM"))
            ctxps = pb.enter_context(tc.tile_pool(name="ctxps", bufs=2, space="PSUM"))

            nc.scalar.dma_start(
                qT[:], a2a_out.rearrange("(kk p) j -> p kk j", p=128))

            for j in range(NB):
                tcj = tcs[j]
                n0 = min(tcj, 16)
                n1_ = tcj - n0
                kT_sb = kvp.tile([128, S], E3, name="kT_sb", tag="kT")
                nc.scalar.dma_start(kT_sb[:, :tcj * 128], kT_i[j][:, :tcj * 128])
                v_sb = kvp.tile([128, 32 * HD], E3, name="v_sb", tag="v")
                nc.scalar.dma_start(v_sb[:, :tcj * 128], v_i[j][:, :tcj * 128])
                sv_col = smb.tile([128, 1], F32, name="sv_col", tag="sv")
                nc.scalar.dma_start(sv_col[:],
                                    seqm1_i[0:1, j:j + 1].to_broadcast((128, 1)))
                vcur = smb.tile([1, HD], BF16, name="vcur", tag="vcur")
                nc.scalar.dma_start(
                    vcur[:],
                    a2a_out.rearrange("q j -> j q")[j:j + 1, 4096:4224])
                vcur_s = smb.tile([1, HD], BF16, name="vcur_s", tag="vcurs")
                nc.scalar.activation(vcur_s[:], vcur[:], AF.Copy, scale=KVV)

                qT_b = qT[:, 0:32, j]          # (128, 32) strided q^T
                s0 = scps.tile([128, 512], F32, name="s0", tag="sc")
                s1 = scps.tile([128, 512], F32, name="s1", tag="sc")
                for c in range(tcj):
                    dst = (s0 if c < 16 else s1)[:, (c % 16) * 32:(c % 16) * 32 + 32]
                    nc.tensor.matmul(dst, kT_sb[:, c * 128:(c + 1) * 128],
                                     qT_b, start=True, stop=True)
                cur = dps.tile([1, 32], F32, name="cur", tag="cur")
                nc.tensor.matmul(cur[:], qT[:, 32, j:j + 1], qT_b,
                                 start=True, stop=True)

                pp = ppp.tile([128, 1024], BF16, name="pp", tag="pp")
                nc.scalar.activation(pp[:, :n0 * 32], s0[:, :n0 * 32], AF.Exp,
                                     scale=exp_scale / KKV)
                if n1_:
                    nc.scalar.activation(pp[:, 512:512 + n1_ * 32],
                                         s1[:, :n1_ * 32], AF.Exp,
                                         scale=exp_scale / KKV)
                pcur = smb.tile([1, 32], BF16, name="pcur", tag="pcur")
                nc.scalar.activation(pcur[:], cur[:], AF.Exp, scale=exp_scale)

                m_all = smb.tile([128, 32], BF16, name="m_all", tag="mall")
                nc.vector.tensor_scalar(m_all[:], iota_sb[:], sv_col[:], None,
                                        op0=ALU.is_lt)
                pp3 = pp[:].rearrange("p (c h) -> p c h", h=32)
                nc.vector.tensor_tensor(
                    pp3[:, 0:tcj], pp3[:, 0:tcj],
                    m_all[:, 0:tcj, None].to_broadcast((128, tcj, 32)),
                    op=ALU.mult)

                dsum = dps.tile([1, 512], F32, name="dsum", tag="dsm")
                nc.tensor.matmul(dsum[:, :n0 * 32], ones_bf[:], pp[:, :n0 * 32],
                                 start=True, stop=(n1_ == 0))
                if n1_:
                    nc.tensor.matmul(dsum[:, :n1_ * 32], ones_bf[:],
                                     pp[:, 512:512 + n1_ * 32],
                                     start=False, stop=True,
                                     skip_group_check=True)
                den = smb.tile([1, 32], F32, name="den", tag="den")
                nc.vector.reduce_sum(
                    den[:],
                    dsum[:, :n0 * 32].rearrange("p (c h) -> p h c", h=32),
                    axis=mybir.AxisListType.X)
                nc.vector.tensor_tensor(den[:], den[:], pcur[:], op=ALU.add)
                rden = smb.tile([1, 32], F32, name="rden", tag="rden")
                nc.vector.reciprocal(rden[:], den[:])
                bc = ctxps.tile([128, 32], F32, name="bc", tag="bc")
                nc.tensor.matmul(bc[:], ones_row[:], rden[:], start=True, stop=True)

                ctx = ctxps.tile([128, 32], F32, name="ctx", tag="ctx")
                for c in range(tcj):
                    nc.tensor.matmul(ctx[:], v_sb[:, c * 128:(c + 1) * 128],
                                     pp3[:, c], start=(c == 0), stop=False)
                nc.tensor.matmul(ctx[:], vcur_s[:], pcur[:],
                                 start=False, stop=True)
                bc_sb = smb.tile([128, 32], F32, name="bc_sb", tag="bcs")
                nc.scalar.activation(bc_sb[:], bc[:], AF.Copy)
                ctn_sb = smb.tile([128, 32], BF16, name="ctn_sb", tag="ctns")
                nc.vector.tensor_tensor(ctn_sb[:], ctx[:], bc_sb[:], op=ALU.mult)
                nc.scalar.dma_start(ctx_b[j], ctn_sb[:])

        # ------------- Phase C: AG ctx -> Wo -> hidden slice -------------
        with ExitStack() as pc:
            sC = pc.enter_context(tc.tile_pool(name="sC", bufs=1))
            wops = pc.enter_context(tc.tile_pool(name="wops", bufs=1, space="PSUM"))
            ptC = pc.enter_context(tc.tile_pool(name="ptC", bufs=2, space="PSUM"))

            nc.gpsimd.collective_compute(
                "AllGather", ALU.bypass, replica_groups=rg,
                ins=[ctx_b.opt()], outs=[agc.opt()],
            )
            nc.scalar.dma_start(ctxA[:], agc.rearrange("c j d h -> d (c j) h"))

            wo_ps = wops.tile([128, 512], F32)
            for k in range(32):
                par = k % 2
                nc.tensor.matmul(wo_ps[64 * par:64 * par + 64, :],
                                 ctxA[:, :, k], wo_sb[:, k * 512:(k + 1) * 512],
                                 start=(k < 2), stop=(k >= 30),
                                 tile_position=(0, 64 * par))
            wo_hi = sC.tile([64, 512], F32)
            nc.scalar.activation(wo_hi[:], wo_ps[64:128, :], AF.Copy)
            wo_f = sC.tile([64, 512], F32)
            nc.vector.tensor_tensor(wo_f[:], wo_ps[0:64, :], wo_hi[:], op=ALU.add)
            wo_s = sC.tile([64, 512], F32)
            nc.scalar.activation(wo_s[:], wo_f[:], AF.Copy, scale=1.0 / kwo)
            hidc = sC.tile([64, 512], F32)
            nc.scalar.dma_start(hidc[:], hidc_i[:])
            nc.vector.tensor_tensor(hsl_f[:], wo_s[:], hidc[:], op=ALU.add)
            # bf16 transposed slices for the x^T path
            hsl_bf = sC.tile([64, 512], BF16)
            nc.vector.tensor_copy(hsl_bf[:], hsl_f[:])
            hslT = sC.tile([128, 4 * 64], BF16)
            for kk in range(4):
                pt = ptC.tile([128, 64], BF16, name="ptC_t", tag="ptC_t")
                nc.tensor.transpose(pt[:], hsl_bf[:, kk * 128:(kk + 1) * 128],
                                    ident64b[:])
                nc.vector.tensor_copy(hslT[:, kk * 64:(kk + 1) * 64], pt[:])
            nc.scalar.dma_start(hslT_b[:].rearrange("k p b -> p k b"),
                                hslT[:].rearrange("p (k b) -> p k b", k=4))
            nc.gpsimd.collective_compute(
                "AllGather", ALU.bypass, replica_groups=rg,
                ins=[hslT_b.opt()], outs=[aghT.opt()],
            )
            # fp32 hidden slices for the exact final residual (overlaps MoE)
            nc.scalar.dma_start(hsl_b[:], hsl_f[:])
            nc.gpsimd.collective_compute(
                "AllGather", ALU.bypass, replica_groups=rg,
                ins=[hsl_b.opt()], outs=[agh.opt()],
            )

        # ------------- Phase D: norm2 -> x^T -> gate -> top2 -------------
        with ExitStack() as pd:
            sD = pd.enter_context(tc.tile_pool(name="sD", bufs=1))
            sqp = pd.enter_context(tc.tile_pool(name="sqp", bufs=2))
            ssps = pd.enter_context(tc.tile_pool(name="ssps", bufs=1, space="PSUM"))
            gps = pd.enter_context(tc.tile_pool(name="gps", bufs=1, space="PSUM"))

            nc.scalar.dma_start(hidTf[:], aghT.rearrange("c k p b -> p (c k) b"))
            ssq2 = ssps.tile([1, 64], F32)
            for k in range(32):
                sq = sqp.tile([128, 64], BF16, name="sq", tag="sq")
                nc.scalar.activation(sq[:], hidTf[:, k], AF.Square)
                nc.tensor.matmul(ssq2[:], ones_bf[:], sq[:],
                                 start=(k == 0), stop=(k == 31))
            rs2 = sD.tile([1, 64], F32)
            nc.scalar.activation(rs2[:], ssq2[:], AF.Sqrt, bias=float(HID) * EPS)
            nc.vector.reciprocal(rs2[:], rs2[:])
            bc2 = ssps.tile([128, 64], F32)
            nc.tensor.matmul(bc2[:], ones_row[:], rs2[:], start=True, stop=True)
            nc.vector.tensor_tensor(
                xT[:], hidTf[:],
                bc2[:, None, :].to_broadcast((128, 32, 64)), op=ALU.mult)

            g_ps = gps.tile([64, 8], F32)
            for k in range(32):
                nc.tensor.matmul(g_ps[:], xT[:, k], gate_sb[:, k * 8:(k + 1) * 8],
                                 start=(k == 0), stop=(k == 31))
            pg = sD.tile([64, 8], F32)
            nc.scalar.activation(pg[:], g_ps[:], AF.Exp)
            m1c = sD.tile([64, 1], F32)
            nc.vector.reduce_max(m1c[:], pg[:], axis=mybir.AxisListType.X)
            eq1 = sD.tile([64, 8], F32)
            nc.vector.tensor_scalar(eq1[:], pg[:], m1c[:], None, op0=ALU.is_ge)
            t1 = sD.tile([64, 8], F32)
            nc.vector.tensor_tensor(t1[:], pg[:], eq1[:], op=ALU.mult)
            nc.vector.tensor_tensor(t1[:], pg[:], t1[:], op=ALU.subtract)
            m2c = sD.tile([64, 1], F32)
            nc.vector.reduce_max(m2c[:], t1[:], axis=mybir.AxisListType.X)
            keep = sD.tile([64, 8], F32)
            nc.vector.tensor_scalar(keep[:], pg[:], m2c[:], None, op0=ALU.is_ge)
            wsum = sD.tile([64, 1], F32)
            nc.vector.tensor_tensor(wsum[:], m1c[:], m2c[:], op=ALU.add)
            nc.vector.reciprocal(wsum[:], wsum[:])
            wts = sD.tile([64, 8], F32)
            nc.vector.tensor_tensor(wts[:], pg[:], keep[:], op=ALU.mult)
            nc.vector.tensor_scalar_mul(wts[:], wts[:], wsum[:])
            nc.vector.tensor_tensor(wts[:], wts[:], sel_bc[:], op=ALU.mult)
            nc.vector.reduce_sum(wsel_col[:], wts[:], axis=mybir.AxisListType.X)

        mid_scope.close()   # frees wo/gate SBUF before the MoE peak

        # ------------- Phase E: MoE expert FFN + AllReduce -------------
        with ExitStack() as pe1:
            gups = pe1.enter_context(tc.tile_pool(name="gups", bufs=1, space="PSUM"))
            ptE = pe1.enter_context(tc.tile_pool(name="ptE", bufs=2, space="PSUM"))
            sE = pe1.enter_context(tc.tile_pool(name="sE", bufs=1))

            gu = gups.tile([128, TWO_MI], F32)
            slices = [(o * 512, min(512, TWO_MI - o * 512)) for o in range(6)]
            for k in range(32):
                par = k % 2
                w1t = w1_tiles[k // 8]
                base = (k % 8) * TWO_MI
                for (off, w) in slices:
                    nc.tensor.matmul(gu[64 * par:64 * par + 64, off:off + w],
                                     xT[:, k],
                                     w1t[:, base + off:base + off + w],
                                     start=(k < 2), stop=(k >= 30),
                                     tile_position=(0, 64 * par))
            gu_hi = sE.tile([64, TWO_MI], F32)
            nc.scalar.activation(gu_hi[:], gu[64:128, :], AF.Copy)
            gusum = sE.tile([64, TWO_MI], BF16)
            nc.vector.tensor_tensor(gusum[:], gu[0:64, :], gu_hi[:], op=ALU.add)
            sg = sE.tile([64, MI], BF16)
            nc.scalar.activation(sg[:], gusum[:, :MI], AF.Silu, scale=1.0 / k1)
            mid = sE.tile([64, MI], BF16)
            nc.vector.tensor_tensor(mid[:], sg[:], gusum[:, MI:], op=ALU.mult)
            for mk in range(11):
                pt = ptE.tile([128, 64], BF16, name="ptE_t", tag="ptE_t")
                nc.tensor.transpose(pt[:], mid[:, mk * 128:(mk + 1) * 128],
                                    ident64b[:])
                nc.vector.tensor_copy(midT[:, mk * 64:(mk + 1) * 64], pt[:])

        with ExitStack() as pe2:
            mops = pe2.enter_context(tc.tile_pool(name="mops", bufs=1, space="PSUM"))
            sF = pe2.enter_context(tc.tile_pool(name="sF", bufs=1))
            mo = mops.tile([128, HID], F32)
            for half in range(2):
                cs = slice(half * 2048, (half + 1) * 2048)
                for mk in range(11):
                    par = mk % 2
                    w2t = w2_tiles[mk // 6]
                    base = (mk % 6) * HID + half * 2048
                    for oc in range(4):
                        nc.tensor.matmul(
                            mo[64 * par:64 * par + 64,
                               half * 2048 + oc * 512:half * 2048 + (oc + 1) * 512],
                            midT[:, mk * 64:(mk + 1) * 64],
                            w2t[:, base + oc * 512:base + (oc + 1) * 512],
                            start=(mk < 2), stop=(mk >= 9),
                            tile_position=(0, 64 * par))
                mo_hi = sF.tile([64, 2048], F32, name="mo_hi", tag="moh")
                nc.scalar.activation(mo_hi[:], mo[64:128, cs], AF.Copy,
                                     scale=wsel_col[:])
                mo_lo = sF.tile([64, 2048], F32, name="mo_lo", tag="mol")
                nc.vector.tensor_scalar_mul(mo_lo[:], mo[0:64, cs], wsel_col[:])
                mo_w = sF.tile([64, 2048], F32, name="mo_w", tag="mow")
                nc.vector.tensor_tensor(mo_w[:], mo_lo[:], mo_hi[:], op=ALU.add)
                nc.scalar.dma_start((moe_b0 if half == 0 else moe_b1)[:], mo_w[:])
                nc.gpsimd.collective_compute(
                    "AllReduce", ALU.add, replica_groups=rg,
                    ins=[(moe_b0 if half == 0 else moe_b1).opt()],
                    outs=[(ar_o0 if half == 0 else ar_o1).opt()],
                )

            hidf = sF.tile([64, HID], F32)
            nc.scalar.dma_start(hidf[:].rearrange("b (c o) -> b c o", c=8),
                                agh.rearrange("c b o -> b c o"))
            for half, ar_o in ((0, ar_o0), (1, ar_o1)):
                cs = slice(half * 2048, (half + 1) * 2048)
                ar_sb = sF.tile([64, 2048], F32, name="ar_sb", tag="ar", bufs=2)
                nc.scalar.dma_start(ar_sb[:], ar_o[:])
                nc.vector.tensor_tensor(hidf[:, cs], ar_sb[:], hidf[:, cs],
                                        op=ALU.add)
                nc.scalar.dma_start(out_o[:, cs], hidf[:, cs])

    nc.compile()
    return nc


_NC_CACHE = None
_CACHE_KEY = None


def kernel(hidden_states, positions, k_cache, v_cache, seq_lens,
           norm1_w, norm2_w, Wqkv, Wo, gate_w, w1, w2):
    global LAST_RESULT, _NC_CACHE, _CACHE_KEY

    hs = np.asarray(hidden_states, np.float32).reshape(B, HID)
    seq = np.asarray(seq_lens, np.int32)
    n1 = np.asarray(norm1_w, np.float32) * 64.0
    n2 = np.asarray(norm2_w, np.float32) * 64.0

    # sort batches by seq_len desc, deal round-robin: core c slot j gets
    # original batch P[8c+j] = order[8j+c]; slot trip count from slot max.
    order = np.argsort(-seq, kind="stable")
    P = np.empty(B, np.int64)
    for j in range(NB):
        for c in range(NC_):
            P[NC_ * c + j] = order[NB * j + c]
    tcs = tuple(int(math.ceil(max(int(seq[order[NB * j]]), 1) / 128.0))
                for j in range(NB))

    hs_p = hs[P]
    wq = np.asarray(Wqkv, np.float32)[:QROWS] * n1[None, :]
    kq = _pow2_scale(wq)
    wo_fold = np.asarray(Wo, np.float32) * (1.0 / KVV)
    kwo = _pow2_scale(wo_fold)
    gT_full = (np.asarray(gate_w, np.float32) * n2[None, :])
    w1n = np.asarray(w1, np.float32) * n2[None, None, :]
    k1 = _pow2_scale(w1n)
    w2f = np.asarray(w2, np.float32)
    k2 = _pow2_scale(w2f)

    key = (tcs, kq, kwo, k1)
    if _NC_CACHE is None or _CACHE_KEY != key:
        _NC_CACHE = _build_program(tcs, kq, kwo, k1)
        _CACHE_KEY = key
    nc = _NC_CACHE

    hidT = np.ascontiguousarray(
        hs_p.T.reshape(32, 128, 64).transpose(1, 0, 2).reshape(128, 32 * 64)
    ).astype(BF)
    iota2d = (np.arange(128, dtype=np.float32)[:, None]
              + 128.0 * np.arange(32, dtype=np.float32)[None, :])
    seqm1_p = (seq[P].astype(np.float32) - 1.0)

    khat = _e3(np.asarray(k_cache, np.float32) * KKV)
    vhat = _e3(np.asarray(v_cache, np.float32) * KVV)

    in_maps = []
    for c in range(NC_):
        Pc = P[c * NB:(c + 1) * NB]
        sel = np.zeros((1, 8), np.float32)
        sel[0, c] = 1.0 / (k1 * k2)
        wq_c = _e3(wq[c * RPC:(c + 1) * RPC] * kq)           # (528, 4096)
        wo_c = _e3(wo_fold[c * 512:(c + 1) * 512] * kwo)     # (512, 4096)
        w1_c = _e3(w1n[c] * k1)                              # (2816, 4096)
        w2_c = _e3(w2f[c] * k2)                              # (4096, 1408)
        in_maps.append({
            "hidT": hidT,
            "hbf": hs_p.astype(BF),
            "hidcols": np.ascontiguousarray(hs_p[:, c * 512:(c + 1) * 512]),
            "wqkvT": _pack32(wq_c.astype(np.float32), 32, RPC).astype(E3M4),
            "woT": _pack32(wo_c.astype(np.float32), 32, 512).astype(E3M4),
            "gateT": _pack32(gT_full, 32, 8).astype(BF),
            "w1T": _pack32(w1_c.astype(np.float32), 32, TWO_MI).astype(E3M4),
            "w2T": _pack32(w2_c.astype(np.float32), 11, HID).astype(E3M4),
            "kT": np.ascontiguousarray(khat[Pc].transpose(0, 2, 1)),
            "v": np.ascontiguousarray(
                vhat[Pc].reshape(NB, 32, 128, HD).transpose(0, 2, 1, 3)
                .reshape(NB, 128, 32 * HD)),
            "seqm1": np.ascontiguousarray(seqm1_p[c * NB:(c + 1) * NB]
                                          .reshape(1, NB)),
            "sel": sel,
            "iota2d": iota2d,
        })

    LAST_RESULT = run_bass_kernel_spmd(nc, in_maps, core_ids=list(range(NC_)))
    res_p = LAST_RESULT.results[0]["out"]
    out = np.empty((B, HID), np.float32)
    out[P] = res_p
    return out.reshape(B, 1, HID).astype(np.float32)


# revision 35
# speedup vs baseline: 2.0211x; 1.1014x over previous
"""DeepSeek-V2 decode layer on 8 TRN2 NeuronCores (Bass/Tile SPMD kernel), v3.

Sharding (per core c of 8):
  - QKV proj row-parallel (512 q-rows/core, e3m4); current-token k/v rows
    (128) replicated on every core. Per-core q^T slots are extracted from an
    AllGather (bf16) of the row shards via one-hot matmuls (no AllToAll, no
    core-dependent addressing).
  - Attention data-parallel over batch: batches sorted by seq_len, dealt
    round-robin so slot j has compile-time trip count tc[j]; KV cache e3m4.
    Scores run 4 slots concurrently in PE column groups (q^T stationary,
    kT streaming, s-blocks of 1024); probs are block-transposed on the DVE,
    masked, then ctx runs 4-slot col-grouped with v+ones moving (the ones
    column yields the softmax denominator for free).
  - ctx -> AllGather (bf16) -> Wo col-parallel (e3m4) -> hidden slice fp32.
    One combined AllGather carries hidden^T (bf16, for the MoE x^T path) and
    hidden (fp32, for the exact final residual, consumed late).
  - MoE expert-parallel dense, w1/w2 e3m4, paired PE column tiling for M=64;
    final AllReduce in bf16, 2 column chunks, overlapped with w2 compute.
Dequant folds: kq^2 into rsqrt(ms), HD^-0.5/kkv into the Exp scale, kvv into
Wo host-side, 1/(k1*k2) into the expert one-hot `sel`, 1/kwo into a copy.
"""

import os
import sys
import math

import numpy as np
import ml_dtypes

for _p in ("/opt/trn_rl_repo", "/root/.axon_site/_ro/trn_rl_repo", "/root/.axon_site"):
    if _p not in sys.path and os.path.isdir(_p):
        sys.path.append(_p)


def _ensure_ntff_hook():
    """This image's antenv lacks axon_hooks; shim it so BASS_TRACE works."""
    import types

    try:
        import antenv.axon_hooks  # noqa: F401
        return
    except ImportError:
        pass
    import antenv

    mod = types.ModuleType("antenv.axon_hooks")
    _state = {"h": None}
    mod.set_axon_ntff_profile_hook = lambda h: _state.__setitem__("h", h)
    mod.get_axon_ntff_profile_hook = lambda: _state["h"]
    sys.modules["antenv.axon_hooks"] = mod
    antenv.axon_hooks = mod
    try:
        sys.path.insert(0, "/root/.axon_site/trn_agent_boot")
        import trn_boot

        so_path = "/opt/axon/libaxon_pjrt.so"
        if os.path.exists(so_path):
            mod.set_axon_ntff_profile_hook(
                trn_boot._ntff_profile_via_ctypes(so_path))
    except Exception as e:  # tracing degrades; compile+run still work
        print(f"ntff hook install failed: {e}")


_ensure_ntff_hook()

import concourse.bacc as bacc
import concourse.bass as bass
import concourse.mybir as mybir
import concourse.tile as tile
from concourse.bass_utils import run_bass_kernel_spmd
from concourse.masks import make_identity
from contextlib import ExitStack

F32 = mybir.dt.float32
BF16 = mybir.dt.bfloat16
E3 = mybir.dt.float8e3
AF = mybir.ActivationFunctionType
ALU = mybir.AluOpType

B, HID, S, NH, HD = 64, 4096, 4096, 32, 128
QROWS = NH * HD + HD          # 4224 used rows of Wqkv (q + current-k)
NB = B // 8                   # 8 batches (slots) per core
MI, TWO_MI = 1408, 2816
NC_ = 8
EPS = 1e-6
E3M4 = ml_dtypes.float8_e3m4
BF = ml_dtypes.bfloat16
KKV = 2.0                     # host scale on k cache
KVV = 2.0                     # host scale on v cache

LAST_RESULT = None            # BassKernelResults of the most recent run


def _pow2_scale(x, target=2.0):
    s = float(np.asarray(x, np.float32).std())
    if s <= 0:
        return 1.0
    return 2.0 ** round(math.log2(target / s))


def _e3(x):
    return np.clip(np.asarray(x, np.float32), -15.0, 15.0).astype(E3M4)


def _pack32(wT, nk, ncols):
    """[ncols, K=nk*128] weight (row-major) -> [128, nk*ncols] chunk-packed:
    pack[p, k*ncols + r] = wT[r, k*128 + p]."""
    return np.ascontiguousarray(
        wT.T.reshape(nk, 128, ncols).transpose(1, 0, 2).reshape(128, nk * ncols))


def _build_program(tcs, kq, kwo, k1):
    nc = bacc.Bacc(None, target_bir_lowering=False, num_devices=NC_)

    hidT_i = nc.dram_tensor("hidT", [128, 32 * 64], BF16, kind="ExternalInput")
    h_i = nc.dram_tensor("hbf", [B, HID], BF16, kind="ExternalInput")
    hidc_i = nc.dram_tensor("hidcols", [B, 512], F32, kind="ExternalInput")
    wqkvT_i = nc.dram_tensor("wqkvT", [128, 32 * 512], E3, kind="ExternalInput")
    wkT_i = nc.dram_tensor("wkT", [128, 32 * 128], E3, kind="ExternalInput")
    woT_i = nc.dram_tensor("woT", [128, 32 * 512], E3, kind="ExternalInput")
    gateT_i = nc.dram_tensor("gateT", [128, 32 * 8], BF16, kind="ExternalInput")
    w1T_i = nc.dram_tensor("w1T", [128, 32 * TWO_MI], E3, kind="ExternalInput")
    w2T_i = nc.dram_tensor("w2T", [128, 11 * HID], E3, kind="ExternalInput")
    kT_i = nc.dram_tensor("kT", [NB, HD, S], E3, kind="ExternalInput")
    vx_i = nc.dram_tensor("vx", [NB, 128, 32 * 129], E3, kind="ExternalInput")
    seqm1_i = nc.dram_tensor("seqm1", [1, NB], F32, kind="ExternalInput")
    sel_i = nc.dram_tensor("sel", [1, 8], F32, kind="ExternalInput")
    sel64_i = nc.dram_tensor("sel64", [B, 8], BF16, kind="ExternalInput")
    iota_i = nc.dram_tensor("iota2d", [128, 32], F32, kind="ExternalInput")
    out_o = nc.dram_tensor("out", [B, HID], F32, kind="ExternalOutput")

    rg = [list(range(NC_))]
    exp_scale = float(HD) ** -0.5
    # combined hidden AllGather payload: 32768 f32 (hsl) + 16384 f32-equiv
    # (hslT bf16) = 49152 f32 per rank
    NCOMBO = 49152

    with tile.TileContext(nc) as tc, ExitStack() as top:
        dramp = top.enter_context(tc.tile_pool(name="dram", bufs=1, space="DRAM"))
        agq_in = dramp.tile([B, 512], BF16)
        agq = dramp.tile([NC_, B, 512], BF16, addr_space="Shared")
        ctx_b = dramp.tile([NB, HD, 32], BF16)
        agc = dramp.tile([NC_, NB, HD, 32], BF16, addr_space="Shared")
        combo = dramp.tile([1, NCOMBO], F32)
        comboag = dramp.tile([NC_, NCOMBO], F32, addr_space="Shared")
        moe_b0 = dramp.tile([B, 2048], BF16)
        moe_b1 = dramp.tile([B, 2048], BF16)
        ar_o0 = dramp.tile([B, 2048], BF16, addr_space="Shared")
        ar_o1 = dramp.tile([B, 2048], BF16, addr_space="Shared")

        const = top.enter_context(tc.tile_pool(name="const", bufs=1))
        ident64b = const.tile([64, 64], BF16)
        make_identity(nc, ident64b)
        ident32b = const.tile([32, 32], BF16)
        make_identity(nc, ident32b)
        ones_bf = const.tile([128, 1], BF16)
        nc.gpsimd.memset(ones_bf[:], 1.0)
        ones_row = const.tile([1, 128], F32)
        nc.gpsimd.memset(ones_row[:], 1.0)
        zero_col = const.tile([128, 1], F32)
        nc.gpsimd.memset(zero_col[:], 0.0)
        nc.const_aps.aps[(F32, 0.0)] = zero_col[:]
        epsq_col = const.tile([128, 1], F32)
        nc.gpsimd.memset(epsq_col[:], float(HID) * EPS * kq * kq)
        nc.const_aps.aps[(F32, float(HID) * EPS * kq * kq)] = epsq_col[:]
        eps_col = const.tile([128, 1], F32)
        nc.gpsimd.memset(eps_col[:], float(HID) * EPS)
        nc.const_aps.aps[(F32, float(HID) * EPS)] = eps_col[:]
        iota_sb = const.tile([128, 32], F32)
        nc.scalar.dma_start(iota_sb[:], iota_i[:])
        sel_bc = const.tile([64, 8], F32)
        nc.scalar.dma_start(sel_bc[:], sel_i.ap().to_broadcast((64, 8)))
        sel64_sb = const.tile([B, 8], BF16)
        nc.scalar.dma_start(sel64_sb[:], sel64_i[:])

        small = top.enter_context(tc.tile_pool(name="small", bufs=1))
        acts = top.enter_context(tc.tile_pool(name="acts", bufs=1))
        qT = acts.tile([128, 33, NB], BF16)        # per-slot q^T (+ kcurT at 32)
        ctxA = acts.tile([128, 64, 32], BF16)
        hidTf = acts.tile([128, 32, 64], BF16)
        xT = acts.tile([128, 32, 64], BF16)
        midT = acts.tile([128, 11 * 64], BF16)
        hsl_f = acts.tile([64, 512], F32)
        vcur1 = acts.tile([1, NB, 129], BF16)      # scaled vcur rows ++ ones col
        vx_cur = acts.tile([128, 129], BF16)       # row 0 = this slot's vcur
        ppT_cur = acts.tile([128, 32], BF16)       # row 0 = this slot's pcur
        nc.gpsimd.memset(vx_cur[:], 0.0)
        nc.gpsimd.memset(ppT_cur[:], 0.0)
        wsel_col = small.tile([64, 1], F32, name="wsel_col")

        # ---------------- Phase A: norm1 -> qkv -> AG -> slot extract --------
        with ExitStack() as pa:
            sA = pa.enter_context(tc.tile_pool(name="sA", bufs=1))
            qkvps = pa.enter_context(tc.tile_pool(name="qkvps", bufs=1, space="PSUM"))
            qtps = pa.enter_context(tc.tile_pool(name="qtps", bufs=2, space="PSUM"))

            wq_sb = sA.tile([128, 32 * 512], E3)
            nc.sync.dma_start(wq_sb[:], wqkvT_i[:])
            wk_sb = sA.tile([128, 32 * 128], E3)
            nc.sync.dma_start(wk_sb[:], wkT_i[:])
            hT = sA.tile([128, 32 * 64], BF16)
            nc.scalar.dma_start(hT[:], hidT_i[:])
            h_sb = sA.tile([B, HID], BF16)
            nc.scalar.dma_start(h_sb[:], h_i[:])
            sq_scr = sA.tile([B, HID], BF16)
            ssq = small.tile([64, 1], F32, name="ssq")
            nc.scalar.activation(sq_scr[:], h_sb[:], AF.Square, accum_out=ssq[:])
            rs_col = small.tile([64, 1], F32, name="rs_col")
            nc.scalar.activation(rs_col[:], ssq[:], AF.Sqrt,
                                 bias=float(HID) * EPS * kq * kq, scale=kq * kq)
            nc.vector.reciprocal(rs_col[:], rs_col[:])

            q1 = qkvps.tile([128, 512], F32, name="q1")
            kc = qkvps.tile([128, 128], F32, name="kc")
            for k in range(32):
                par = k % 2
                tp = (0, 64 * par)
                hk = hT[:, k * 64:(k + 1) * 64]
                nc.tensor.matmul(q1[64 * par:64 * par + 64, :], hk,
                                 wq_sb[:, k * 512:(k + 1) * 512],
                                 start=(k < 2), stop=(k >= 30), tile_position=tp)
                nc.tensor.matmul(kc[64 * par:64 * par + 64, :], hk,
                                 wk_sb[:, k * 128:(k + 1) * 128],
                                 start=(k < 2), stop=(k >= 30), tile_position=tp)
            qkv_hi = sA.tile([64, 512 + 128], F32)
            nc.scalar.activation(qkv_hi[:, :512], q1[64:128, :], AF.Copy)
            nc.scalar.activation(qkv_hi[:, 512:], kc[64:128, :], AF.Copy)
            qkv_f = sA.tile([64, 512 + 128], F32)
            nc.vector.tensor_tensor(qkv_f[:, :512], q1[0:64, :],
                                    qkv_hi[:, :512], op=ALU.add)
            nc.vector.tensor_tensor(qkv_f[:, 512:], kc[0:64, :],
                                    qkv_hi[:, 512:], op=ALU.add)
            qkv_sb = sA.tile([64, 512], BF16)
            nc.vector.tensor_scalar_mul(qkv_sb[:], qkv_f[:, :512], rs_col[:])
            kcur_sb = sA.tile([64, 128], BF16)
            nc.vector.tensor_scalar_mul(kcur_sb[:], qkv_f[:, 512:], rs_col[:])

            nc.scalar.dma_start(agq_in[:], qkv_sb[:])
            nc.gpsimd.collective_compute(
                "AllGather", ALU.bypass, replica_groups=rg,
                ins=[agq_in.opt()], outs=[agq.opt()],
            )
            qag_sb = sA.tile([64, 4096], BF16)
            nc.scalar.dma_start(
                qag_sb[:].rearrange("b (c r) -> b c r", c=8),
                agq.rearrange("c b r -> b c r"))
            # extract my 8 slots' q^T (and kcur^T) via one-hot matmuls
            for kk in range(33):
                lhs = (qag_sb[:, kk * 128:(kk + 1) * 128] if kk < 32
                       else kcur_sb[:])
                qt_ps = qtps.tile([128, 8], F32, name="qt_ps", tag="qt")
                for hh in range(2):
                    nc.tensor.matmul(qt_ps[64 * hh:64 * hh + 64, :],
                                     lhs[:, 64 * hh:64 * hh + 64], sel64_sb[:],
                                     start=True, stop=True,
                                     tile_position=(0, 64 * hh))
                nc.vector.tensor_copy(qT[:, kk, :], qt_ps[:])
            # vcur rows for my slots (scaled by KVV), with a ones column
            nc.gpsimd.memset(vcur1[:, :, 128:129], 1.0)
            for j in range(NB):
                vr = qtps.tile([1, 128], F32, name="vr", tag="vr")
                nc.tensor.matmul(vr[:], sel64_sb[:, j:j + 1], kcur_sb[:],
                                 start=True, stop=True)
                nc.scalar.activation(vcur1[0:1, j, 0:128], vr[:], AF.Copy,
                                     scale=KVV)

        # ------- bulk weight streams (SP queue, consumption order) -------
        w1p = top.enter_context(tc.tile_pool(name="w1p", bufs=3))
        w2p = top.enter_context(tc.tile_pool(name="w2p", bufs=2))
        mid_scope = top.enter_context(ExitStack())
        wstr = mid_scope.enter_context(tc.tile_pool(name="wstr", bufs=1))
        wo_sb = wstr.tile([128, 32 * 512], E3)
        nc.sync.dma_start(wo_sb[:], woT_i[:])
        gate_sb = wstr.tile([128, 32 * 8], BF16)
        nc.sync.dma_start(gate_sb[:], gateT_i[:])
        w1_tiles = [w1p.tile([128, 8 * TWO_MI], E3, name="w1t", tag="w1")
                    for _ in range(4)]
        w2_tiles = [w2p.tile([128, 4 * HID], E3, name="w2t", tag="w2")
                    for _ in range(3)]
        for t in range(3):
            nc.sync.dma_start(w1_tiles[t][:],
                              w1T_i[:, t * 8 * TWO_MI:(t + 1) * 8 * TWO_MI])
        for t, nmk in ((0, 4), (1, 4)):
            nc.sync.dma_start(w2_tiles[t][:, :nmk * HID],
                              w2T_i[:, t * 4 * HID:t * 4 * HID + nmk * HID])
        nc.sync.dma_start(w1_tiles[3][:], w1T_i[:, 3 * 8 * TWO_MI:4 * 8 * TWO_MI])
        nc.sync.dma_start(w2_tiles[2][:, :3 * HID],
                          w2T_i[:, 8 * HID:11 * HID])

        # ---------------- Phase B: attention (2 rounds x 4 slots) ------------
        with ExitStack() as pb:
            kkp = pb.enter_context(tc.tile_pool(name="kkp", bufs=8))
            kvp = pb.enter_context(tc.tile_pool(name="kvp", bufs=8))
            ppp = pb.enter_context(tc.tile_pool(name="ppp", bufs=2))
            ptp = pb.enter_context(tc.tile_pool(name="ptp", bufs=5))
            smb = pb.enter_context(tc.tile_pool(name="smb", bufs=4))
            scps = pb.enter_context(tc.tile_pool(name="scps", bufs=2, space="PSUM"))
            ctxps = pb.enter_context(tc.tile_pool(name="ctxps", bufs=2, space="PSUM"))
            ptB = pb.enter_context(tc.tile_pool(name="ptB", bufs=2, space="PSUM"))
            dps = pb.enter_context(tc.tile_pool(name="dps", bufs=2, space="PSUM"))

            sv_cols = []
            for j in range(NB):
                sv_col = smb.tile([128, 1], F32, name="sv_col", tag="sv", bufs=8)
                nc.scalar.dma_start(
                    sv_col[:], seqm1_i[0:1, j:j + 1].to_broadcast((128, 1)))
                sv_cols.append(sv_col)

            for r in range(2):
                slots = [4 * r + j4 for j4 in range(4)]
                rtcs = [tcs[j] for j in slots]
                nblk = (max(rtcs) * 128 + 1023) // 1024

                pp = ppp.tile([128, 4096], BF16, name="pp", tag="pp")
                # scores: 4 slots concurrent in PE col groups; KV pieces of
                # 1024, matmuls/exp in 512-col sub-blocks (PSUM bank limit)
                vpieces = {}
                for blk in range(nblk):
                    kpieces = {}
                    for j4, j in enumerate(slots):
                        w = min(1024, rtcs[j4] * 128 - blk * 1024)
                        if w <= 0:
                            continue
                        kt = kkp.tile([128, 1024], E3, name="kt", tag="kt")
                        nc.scalar.dma_start(kt[:, :w],
                                            kT_i[j][:, blk * 1024:blk * 1024 + w])
                        kpieces[j4] = kt
                    for sub in range(2):
                        base = blk * 1024 + sub * 512
                        if all(rtcs[j4] * 128 <= base for j4 in range(4)):
                            continue
                        sc = scps.tile([128, 512], F32, name="sc", tag="sc")
                        for j4, j in enumerate(slots):
                            w = min(512, rtcs[j4] * 128 - base)
                            if w <= 0:
                                continue
                            nc.tensor.matmul(
                                sc[32 * j4:32 * j4 + 32, :w],
                                qT[:, 0:32, j],
                                kpieces[j4][:, sub * 512:sub * 512 + w],
                                start=True, stop=True,
                                tile_position=(0, 32 * j4))
                        for j4 in range(4):
                            w = min(512, rtcs[j4] * 128 - base)
                            if w <= 0:
                                continue
                            nc.scalar.activation(
                                pp[32 * j4:32 * j4 + 32, base:base + w],
                                sc[32 * j4:32 * j4 + 32, :w], AF.Exp,
                                scale=exp_scale / KKV)
                ctx = ctxps.tile([128, 129], F32, name="ctx", tag="ctx")
                for j4, j in enumerate(slots):
                    tcj = rtcs[j4]
                    for b8 in range((tcj + 7) // 8):
                        wv = min(8 * 129, tcj * 129 - b8 * 8 * 129)
                        vt = kvp.tile([128, 8 * 129], E3, name="vt", tag="vt")
                        nc.scalar.dma_start(
                            vt[:, :wv], vx_i[j][:, b8 * 8 * 129:b8 * 8 * 129 + wv])
                        vpieces[(j4, b8)] = vt
                    # block-transpose probs into [s, (c, h)] layout, then mask
                    ppT = ptp.tile([128, 32, 32], BF16, name="ppT", tag="ppT")
                    for g in range(4):
                        nc.vector.transpose(
                            ppT[32 * g:32 * g + 32, :tcj, :],
                            pp[32 * j4:32 * j4 + 32, :]
                            .rearrange("h (c g d) -> h c (g d)", g=4, d=32)
                            [:, :tcj, 32 * g:32 * g + 32])
                    m_all = smb.tile([128, 32], BF16, name="m_all", tag="mall")
                    nc.vector.tensor_scalar(m_all[:], iota_sb[:], sv_cols[j][:],
                                            None, op0=ALU.is_lt)
                    nc.vector.tensor_tensor(
                        ppT[:, :tcj, :], ppT[:, :tcj, :],
                        m_all[:, :tcj, None].to_broadcast((128, tcj, 32)),
                        op=ALU.mult)
                    # current-token prob
                    cur = dps.tile([1, 32], F32, name="cur", tag="cur")
                    nc.tensor.matmul(cur[:], qT[:, 32, j:j + 1], qT[:, 0:32, j],
                                     start=True, stop=True)
                    nc.scalar.activation(ppT_cur[0:1, :], cur[:], AF.Exp,
                                         scale=exp_scale)
                    nc.vector.tensor_copy(vx_cur[0:1, :], vcur1[0:1, j, :])
                    # ctx (+ den in col 128 via the ones column of vx)
                    for c in range(tcj):
                        nc.tensor.matmul(
                            ctx[32 * j4:32 * j4 + 32, :],
                            ppT[:, c, :],
                            vpieces[(j4, c // 8)][:, (c % 8) * 129:(c % 8) * 129 + 129],
                            start=(c == 0), stop=False,
                            tile_position=(0, 32 * j4))
                    nc.tensor.matmul(ctx[32 * j4:32 * j4 + 32, :],
                                     ppT_cur[:], vx_cur[:],
                                     start=False, stop=True,
                                     tile_position=(0, 32 * j4))
                    den = smb.tile([32, 1], F32, name="den", tag="den")
                    nc.scalar.activation(den[:], ctx[32 * j4:32 * j4 + 32, 128:129],
                                         AF.Copy)
                    rden = smb.tile([32, 1], F32, name="rden", tag="rden")
                    nc.vector.reciprocal(rden[:], den[:])
                    ctn = smb.tile([32, 128], BF16, name="ctn", tag="ctn")
                    nc.vector.tensor_scalar_mul(
                        ctn[:], ctx[32 * j4:32 * j4 + 32, 0:128], rden[:])
                    ctnT = ptB.tile([128, 32], BF16, name="ctnT", tag="ctnT")
                    nc.tensor.transpose(ctnT[:], ctn[:], ident32b[:])
                    ctnT_sb = smb.tile([128, 32], BF16, name="ctnT_sb", tag="cts")
                    nc.vector.tensor_copy(ctnT_sb[:], ctnT[:])
                    nc.scalar.dma_start(ctx_b[j], ctnT_sb[:])

        # ------------- Phase C: AG ctx -> Wo -> hidden slice -------------
        with ExitStack() as pc:
            sC = pc.enter_context(tc.tile_pool(name="sC", bufs=1))
            wops = pc.enter_context(tc.tile_pool(name="wops", bufs=1, space="PSUM"))
            ptC = pc.enter_context(tc.tile_pool(name="ptC", bufs=2, space="PSUM"))

            nc.gpsimd.collective_compute(
                "AllGather", ALU.bypass, replica_groups=rg,
                ins=[ctx_b.opt()], outs=[agc.opt()],
            )
            nc.scalar.dma_start(ctxA[:], agc.rearrange("c j d h -> d (c j) h"))

            wo_ps = wops.tile([128, 512], F32)
            for k in range(32):
                par = k % 2
                nc.tensor.matmul(wo_ps[64 * par:64 * par + 64, :],
                                 ctxA[:, :, k], wo_sb[:, k * 512:(k + 1) * 512],
                                 start=(k < 2), stop=(k >= 30),
                                 tile_position=(0, 64 * par))
            wo_hi = sC.tile([64, 512], F32)
            nc.scalar.activation(wo_hi[:], wo_ps[64:128, :], AF.Copy)
            wo_f = sC.tile([64, 512], F32)
            nc.vector.tensor_tensor(wo_f[:], wo_ps[0:64, :], wo_hi[:], op=ALU.add)
            wo_s = sC.tile([64, 512], F32)
            nc.scalar.activation(wo_s[:], wo_f[:], AF.Copy, scale=1.0 / kwo)
            hidc = sC.tile([64, 512], F32)
            nc.scalar.dma_start(hidc[:], hidc_i[:])
            nc.vector.tensor_tensor(hsl_f[:], wo_s[:], hidc[:], op=ALU.add)
            # combined AG payload: hslT (bf16) for x^T + hsl (fp32) residual
            hsl_bf = sC.tile([64, 512], BF16)
            nc.vector.tensor_copy(hsl_bf[:], hsl_f[:])
            hslT = sC.tile([128, 4 * 64], BF16)
            for kk in range(4):
                pt = ptC.tile([128, 64], BF16, name="ptC_t", tag="ptC_t")
                nc.tensor.transpose(pt[:], hsl_bf[:, kk * 128:(kk + 1) * 128],
                                    ident64b[:])
                nc.vector.tensor_copy(hslT[:, kk * 64:(kk + 1) * 64], pt[:])
            nc.scalar.dma_start(
                combo[0, :32768].rearrange("(b o) -> b o", b=64), hsl_f[:])
            nc.scalar.dma_start(
                combo[0, 32768:].bitcast(BF16).rearrange("(p f) -> p f", p=128),
                hslT[:])
            nc.gpsimd.collective_compute(
                "AllGather", ALU.bypass, replica_groups=rg,
                ins=[combo.opt()], outs=[comboag.opt()],
            )

        # ------------- Phase D: norm2 -> x^T -> gate -> top2 -------------
        with ExitStack() as pd:
            sD = pd.enter_context(tc.tile_pool(name="sD", bufs=1))
            sqp = pd.enter_context(tc.tile_pool(name="sqp", bufs=2))
            ssps = pd.enter_context(tc.tile_pool(name="ssps", bufs=1, space="PSUM"))
            gps = pd.enter_context(tc.tile_pool(name="gps", bufs=1, space="PSUM"))

            nc.scalar.dma_start(
                hidTf[:].rearrange("p (c k) b -> p c k b", c=8),
                comboag[:, 32768:].bitcast(BF16)
                .rearrange("c (p k b) -> p c k b", p=128, k=4))
            ssq2 = ssps.tile([1, 64], F32)
            for k in range(32):
                sq = sqp.tile([128, 64], BF16, name="sq", tag="sq")
                nc.scalar.activation(sq[:], hidTf[:, k], AF.Square)
                nc.tensor.matmul(ssq2[:], ones_bf[:], sq[:],
                                 start=(k == 0), stop=(k == 31))
            rs2 = sD.tile([1, 64], F32)
            nc.scalar.activation(rs2[:], ssq2[:], AF.Sqrt, bias=float(HID) * EPS)
            nc.vector.reciprocal(rs2[:], rs2[:])
            bc2 = ssps.tile([128, 64], F32)
            nc.tensor.matmul(bc2[:], ones_row[:], rs2[:], start=True, stop=True)
            nc.vector.tensor_tensor(
                xT[:], hidTf[:],
                bc2[:, None, :].to_broadcast((128, 32, 64)), op=ALU.mult)

            g_ps = gps.tile([64, 8], F32)
            for k in range(32):
                nc.tensor.matmul(g_ps[:], xT[:, k], gate_sb[:, k * 8:(k + 1) * 8],
                                 start=(k == 0), stop=(k == 31))
            pg = sD.tile([64, 8], F32)
            nc.scalar.activation(pg[:], g_ps[:], AF.Exp)
            m1c = sD.tile([64, 1], F32)
            nc.vector.reduce_max(m1c[:], pg[:], axis=mybir.AxisListType.X)
            eq1 = sD.tile([64, 8], F32)
            nc.vector.tensor_scalar(eq1[:], pg[:], m1c[:], None, op0=ALU.is_ge)
            t1 = sD.tile([64, 8], F32)
            nc.vector.tensor_tensor(t1[:], pg[:], eq1[:], op=ALU.mult)
            nc.vector.tensor_tensor(t1[:], pg[:], t1[:], op=ALU.subtract)
            m2c = sD.tile([64, 1], F32)
            nc.vector.reduce_max(m2c[:], t1[:], axis=mybir.AxisListType.X)
            keep = sD.tile([64, 8], F32)
            nc.vector.tensor_scalar(keep[:], pg[:], m2c[:], None, op0=ALU.is_ge)
            wsum = sD.tile([64, 1], F32)
            nc.vector.tensor_tensor(wsum[:], m1c[:], m2c[:], op=ALU.add)
            nc.vector.reciprocal(wsum[:], wsum[:])
            wts = sD.tile([64, 8], F32)
            nc.vector.tensor_tensor(wts[:], pg[:], keep[:], op=ALU.mult)
            nc.vector.tensor_scalar_mul(wts[:], wts[:], wsum[:])
            nc.vector.tensor_tensor(wts[:], wts[:], sel_bc[:], op=ALU.mult)
            nc.vector.reduce_sum(wsel_col[:], wts[:], axis=mybir.AxisListType.X)

        mid_scope.close()   # frees wo/gate SBUF before the MoE peak

        # ------------- Phase E: MoE expert FFN + AllReduce -------------
        with ExitStack() as pe1:
            gups = pe1.enter_context(tc.tile_pool(name="gups", bufs=1, space="PSUM"))
            ptE = pe1.enter_context(tc.tile_pool(name="ptE", bufs=2, space="PSUM"))
            sE = pe1.enter_context(tc.tile_pool(name="sE", bufs=1))

            gu = gups.tile([128, TWO_MI], F32)
            slices = [(o * 512, min(512, TWO_MI - o * 512)) for o in range(6)]
            for k in range(32):
                par = k % 2
                w1t = w1_tiles[k // 8]
                base = (k % 8) * TWO_MI
                for (off, w) in slices:
                    nc.tensor.matmul(gu[64 * par:64 * par + 64, off:off + w],
                                     xT[:, k],
                                     w1t[:, base + off:base + off + w],
                                     start=(k < 2), stop=(k >= 30),
                                     tile_position=(0, 64 * par))
            gu_hi = sE.tile([64, TWO_MI], F32)
            nc.scalar.activation(gu_hi[:], gu[64:128, :], AF.Copy)
            gusum = sE.tile([64, TWO_MI], BF16)
            nc.vector.tensor_tensor(gusum[:], gu[0:64, :], gu_hi[:], op=ALU.add)
            sg = sE.tile([64, MI], BF16)
            nc.scalar.activation(sg[:], gusum[:, :MI], AF.Silu, scale=1.0 / k1)
            mid = sE.tile([64, MI], BF16)
            nc.vector.tensor_tensor(mid[:], sg[:], gusum[:, MI:], op=ALU.mult)
            for mk in range(11):
                pt = ptE.tile([128, 64], BF16, name="ptE_t", tag="ptE_t")
                nc.tensor.transpose(pt[:], mid[:, mk * 128:(mk + 1) * 128],
                                    ident64b[:])
                nc.vector.tensor_copy(midT[:, mk * 64:(mk + 1) * 64], pt[:])

        with ExitStack() as pe2:
            mops = pe2.enter_context(tc.tile_pool(name="mops", bufs=1, space="PSUM"))
            sF = pe2.enter_context(tc.tile_pool(name="sF", bufs=1))
            mo = mops.tile([128, HID], F32)
            for half in range(2):
                cs = slice(half * 2048, (half + 1) * 2048)
                for mk in range(11):
                    par = mk % 2
                    w2t = w2_tiles[mk // 4]
                    base = (mk % 4) * HID + half * 2048
                    for oc in range(4):
                        nc.tensor.matmul(
                            mo[64 * par:64 * par + 64,
                               half * 2048 + oc * 512:half * 2048 + (oc + 1) * 512],
                            midT[:, mk * 64:(mk + 1) * 64],
                            w2t[:, base + oc * 512:base + (oc + 1) * 512],
                            start=(mk < 2), stop=(mk >= 9),
                            tile_position=(0, 64 * par))
                mo_hi = sF.tile([64, 2048], F32, name="mo_hi", tag="moh")
                nc.scalar.activation(mo_hi[:], mo[64:128, cs], AF.Copy,
                                     scale=wsel_col[:])
                mo_lo = sF.tile([64, 2048], F32, name="mo_lo", tag="mol")
                nc.vector.tensor_scalar_mul(mo_lo[:], mo[0:64, cs], wsel_col[:])
                mo_w = sF.tile([64, 2048], BF16, name="mo_w", tag="mow")
                nc.vector.tensor_tensor(mo_w[:], mo_lo[:], mo_hi[:], op=ALU.add)
                nc.scalar.dma_start((moe_b0 if half == 0 else moe_b1)[:], mo_w[:])
                nc.gpsimd.collective_compute(
                    "AllReduce", ALU.add, replica_groups=rg,
                    ins=[(moe_b0 if half == 0 else moe_b1).opt()],
                    outs=[(ar_o0 if half == 0 else ar_o1).opt()],
                )

            hidf = sF.tile([64, HID], F32)
            nc.scalar.dma_start(
                hidf[:].rearrange("b (c o) -> b c o", c=8),
                comboag[:, :32768].rearrange("c (b o) -> b c o", b=64))
            for half, ar_o in ((0, ar_o0), (1, ar_o1)):
                cs = slice(half * 2048, (half + 1) * 2048)
                ar_sb = sF.tile([64, 2048], BF16, name="ar_sb", tag="ar", bufs=2)
                nc.scalar.dma_start(ar_sb[:], ar_o[:])
                nc.vector.tensor_tensor(hidf[:, cs], ar_sb[:], hidf[:, cs],
                                        op=ALU.add)
                nc.scalar.dma_start(out_o[:, cs], hidf[:, cs])

    nc.compile()
    return nc


_NC_CACHE = None
_CACHE_KEY = None


def kernel(hidden_states, positions, k_cache, v_cache, seq_lens,
           norm1_w, norm2_w, Wqkv, Wo, gate_w, w1, w2):
    global LAST_RESULT, _NC_CACHE, _CACHE_KEY

    hs = np.asarray(hidden_states, np.float32).reshape(B, HID)
    seq = np.asarray(seq_lens, np.int32)
    n1 = np.asarray(norm1_w, np.float32) * 64.0
    n2 = np.asarray(norm2_w, np.float32) * 64.0

    # sort batches by seq_len desc, deal round-robin: core c slot j gets
    # original batch P[8c+j] = order[8j+c]; slot trip count from slot max.
    order = np.argsort(-seq, kind="stable")
    P = np.empty(B, np.int64)
    for j in range(NB):
        for c in range(NC_):
            P[NC_ * c + j] = order[NB * j + c]
    tcs = tuple(int(math.ceil(max(int(seq[order[NB * j]]), 1) / 128.0))
                for j in range(NB))

    hs_p = hs[P]
    wq = np.asarray(Wqkv, np.float32)[:QROWS] * n1[None, :]
    kq = _pow2_scale(wq)
    wo_fold = np.asarray(Wo, np.float32) * (1.0 / KVV)
    kwo = _pow2_scale(wo_fold)
    gT_full = (np.asarray(gate_w, np.float32) * n2[None, :])
    w1n = np.asarray(w1, np.float32) * n2[None, None, :]
    k1 = _pow2_scale(w1n)
    w2f = np.asarray(w2, np.float32)
    k2 = _pow2_scale(w2f)

    key = (tcs, kq, kwo, k1)
    if _NC_CACHE is None or _CACHE_KEY != key:
        _NC_CACHE = _build_program(tcs, kq, kwo, k1)
        _CACHE_KEY = key
    nc = _NC_CACHE

    hidT = np.ascontiguousarray(
        hs_p.T.reshape(32, 128, 64).transpose(1, 0, 2).reshape(128, 32 * 64)
    ).astype(BF)
    iota2d = (np.arange(128, dtype=np.float32)[:, None]
              + 128.0 * np.arange(32, dtype=np.float32)[None, :])
    seqm1_p = (seq[P].astype(np.float32) - 1.0)

    khat = _e3(np.asarray(k_cache, np.float32) * KKV)
    vhat = _e3(np.asarray(v_cache, np.float32) * KVV)
    wk_pack = _pack32(_e3(wq[4096:QROWS] * kq).astype(np.float32), 32, 128
                      ).astype(E3M4)
    gate_pack = _pack32(gT_full, 32, 8).astype(BF)
    ones129 = np.ones((NB, 128, 32, 1), E3M4)

    in_maps = []
    for c in range(NC_):
        Pc = P[c * NB:(c + 1) * NB]
        sel = np.zeros((1, 8), np.float32)
        sel[0, c] = 1.0 / (k1 * k2)
        sel64 = np.zeros((B, 8), BF)
        for j in range(NB):
            sel64[NB * c + j, j] = 1.0
        wq_c = _e3(wq[c * 512:(c + 1) * 512] * kq)           # (512, 4096)
        wo_c = _e3(wo_fold[c * 512:(c + 1) * 512] * kwo)     # (512, 4096)
        w1_c = _e3(w1n[c] * k1)                              # (2816, 4096)
        w2_c = _e3(w2f[c] * k2)                              # (4096, 1408)
        vv = vhat[Pc].reshape(NB, 32, 128, HD).transpose(0, 2, 1, 3)
        vx = np.concatenate([vv, ones129], axis=3).reshape(NB, 128, 32 * 129)
        in_maps.append({
            "hidT": hidT,
            "hbf": hs_p.astype(BF),
            "hidcols": np.ascontiguousarray(hs_p[:, c * 512:(c + 1) * 512]),
            "wqkvT": _pack32(wq_c.astype(np.float32), 32, 512).astype(E3M4),
            "wkT": wk_pack,
            "woT": _pack32(wo_c.astype(np.float32), 32, 512).astype(E3M4),
            "gateT": gate_pack,
            "w1T": _pack32(w1_c.astype(np.float32), 32, TWO_MI).astype(E3M4),
            "w2T": _pack32(w2_c.astype(np.float32), 11, HID).astype(E3M4),
            "kT": np.ascontiguousarray(khat[Pc].transpose(0, 2, 1)),
            "vx": np.ascontiguousarray(vx),
            "seqm1": np.ascontiguousarray(seqm1_p[c * NB:(c + 1) * NB]
                                          .reshape(1, NB)),
            "sel": sel,
            "sel64": sel64,
            "iota2d": iota2d,
        })

    LAST_RESULT = run_bass_kernel_spmd(nc, in_maps, core_ids=list(range(NC_)))
    res_p = LAST_RESULT.results[0]["out"]
    out = np.empty((B, HID), np.float32)
    out[P] = res_p
    return out.reshape(B, 1, HID).astype(np.float32)


# revision 43
# speedup vs baseline: 2.0896x; 1.0339x over previous
"""DeepSeek-V2 decode layer on 8 TRN2 NeuronCores (Bass/Tile SPMD kernel), v3.

Sharding (per core c of 8):
  - QKV proj row-parallel (512 q-rows/core, e3m4); current-token k/v rows
    (128) replicated on every core. Per-core q^T slots are extracted from an
    AllGather (bf16) of the row shards via one-hot matmuls (no AllToAll, no
    core-dependent addressing).
  - Attention data-parallel over batch: batches sorted by seq_len, dealt
    round-robin so slot j has compile-time trip count tc[j]; KV cache e3m4.
    Scores run 4 slots concurrently in PE column groups (q^T stationary,
    kT streaming, s-blocks of 1024); probs are block-transposed on the DVE,
    masked, then ctx runs 4-slot col-grouped with v+ones moving (the ones
    column yields the softmax denominator for free).
  - ctx -> AllGather (bf16) -> Wo col-parallel (e3m4) -> hidden slice fp32.
    One combined AllGather carries hidden^T (bf16, for the MoE x^T path) and
    hidden (fp32, for the exact final residual, consumed late).
  - MoE expert-parallel dense, w1/w2 e3m4, paired PE column tiling for M=64;
    final AllReduce in bf16, 2 column chunks, overlapped with w2 compute.
Dequant folds: kq^2 into rsqrt(ms), HD^-0.5/kkv into the Exp scale, kvv into
Wo host-side, 1/(k1*k2) into the expert one-hot `sel`, 1/kwo into a copy.
"""

import os
import sys
import math

import numpy as np
import ml_dtypes

for _p in ("/opt/trn_rl_repo", "/root/.axon_site/_ro/trn_rl_repo", "/root/.axon_site"):
    if _p not in sys.path and os.path.isdir(_p):
        sys.path.append(_p)


def _ensure_ntff_hook():
    """This image's antenv lacks axon_hooks; shim it so BASS_TRACE works."""
    import types

    try:
        import antenv.axon_hooks  # noqa: F401
        return
    except ImportError:
        pass
    import antenv

    mod = types.ModuleType("antenv.axon_hooks")
    _state = {"h": None}
    mod.set_axon_ntff_profile_hook = lambda h: _state.__setitem__("h", h)
    mod.get_axon_ntff_profile_hook = lambda: _state["h"]
    sys.modules["antenv.axon_hooks"] = mod
    antenv.axon_hooks = mod
    try:
        sys.path.insert(0, "/root/.axon_site/trn_agent_boot")
        import trn_boot

        so_path = "/opt/axon/libaxon_pjrt.so"
        if os.path.exists(so_path):
            mod.set_axon_ntff_profile_hook(
                trn_boot._ntff_profile_via_ctypes(so_path))
    except Exception as e:  # tracing degrades; compile+run still work
        print(f"ntff hook install failed: {e}")


_ensure_ntff_hook()

import concourse.bacc as bacc
import concourse.bass as bass
import concourse.mybir as mybir
import concourse.tile as tile
from concourse.bass_utils import run_bass_kernel_spmd
from concourse.masks import make_identity
from contextlib import ExitStack

F32 = mybir.dt.float32
BF16 = mybir.dt.bfloat16
E3 = mybir.dt.float8e3
AF = mybir.ActivationFunctionType
ALU = mybir.AluOpType

B, HID, S, NH, HD = 64, 4096, 4096, 32, 128
QROWS = NH * HD + HD          # 4224 used rows of Wqkv (q + current-k)
NB = B // 8                   # 8 batches (slots) per core
MI, TWO_MI = 1408, 2816
NC_ = 8
EPS = 1e-6
E3M4 = ml_dtypes.float8_e3m4
BF = ml_dtypes.bfloat16
KKV = 2.0                     # host scale on k cache
KVV = 2.0                     # host scale on v cache

LAST_RESULT = None            # BassKernelResults of the most recent run


def _pow2_scale(x, target=2.0):
    s = float(np.asarray(x, np.float32).std())
    if s <= 0:
        return 1.0
    return 2.0 ** round(math.log2(target / s))


def _e3(x):
    return np.clip(np.asarray(x, np.float32), -15.0, 15.0).astype(E3M4)


def _pack32(wT, nk, ncols):
    """[ncols, K=nk*128] weight (row-major) -> [128, nk*ncols] chunk-packed:
    pack[p, k*ncols + r] = wT[r, k*128 + p]."""
    return np.ascontiguousarray(
        wT.T.reshape(nk, 128, ncols).transpose(1, 0, 2).reshape(128, nk * ncols))


def _build_program(tcs, kq, kwo, k1):
    nc = bacc.Bacc(None, target_bir_lowering=False, num_devices=NC_)

    hidT_i = nc.dram_tensor("hidT", [128, 32 * 64], BF16, kind="ExternalInput")
    h_i = nc.dram_tensor("hbf", [B, HID], BF16, kind="ExternalInput")
    hidc_i = nc.dram_tensor("hidcols", [B, 512], F32, kind="ExternalInput")
    wqkvT_i = nc.dram_tensor("wqkvT", [128, 32 * 512], E3, kind="ExternalInput")
    wkT_i = nc.dram_tensor("wkT", [128, 32 * 128], E3, kind="ExternalInput")
    woT_i = nc.dram_tensor("woT", [128, 32 * 512], E3, kind="ExternalInput")
    gateT_i = nc.dram_tensor("gateT", [128, 32 * 8], BF16, kind="ExternalInput")
    w1T_i = nc.dram_tensor("w1T", [128, 32 * TWO_MI], E3, kind="ExternalInput")
    w2T_i = nc.dram_tensor("w2T", [128, 11 * HID], E3, kind="ExternalInput")
    kT_i = nc.dram_tensor("kT", [NB, HD, S], E3, kind="ExternalInput")
    vx_i = nc.dram_tensor("vx", [NB, 128, 32 * 129], E3, kind="ExternalInput")
    seqm1_i = nc.dram_tensor("seqm1", [1, NB], F32, kind="ExternalInput")
    sel_i = nc.dram_tensor("sel", [1, 8], F32, kind="ExternalInput")
    sel64_i = nc.dram_tensor("sel64", [B, 8], BF16, kind="ExternalInput")
    iota_i = nc.dram_tensor("iota2d", [128, 32], F32, kind="ExternalInput")
    out_o = nc.dram_tensor("out", [B, HID], F32, kind="ExternalOutput")

    rg = [list(range(NC_))]
    exp_scale = float(HD) ** -0.5
    # combined hidden AllGather payload (all bf16): hsl (32768) ++ hslT
    # (32768) ++ per-core rms partial sums (64)
    NCOMBO = 65600

    with tile.TileContext(nc) as tc, ExitStack() as top:
        dramp = top.enter_context(tc.tile_pool(name="dram", bufs=1, space="DRAM"))
        agq_in = dramp.tile([B, 512], BF16)
        agq = dramp.tile([NC_, B, 512], BF16, addr_space="Shared")
        ctx_b = dramp.tile([HD, NB, 32], BF16)
        agc = dramp.tile([NC_, HD, NB, 32], BF16, addr_space="Shared")
        combo = dramp.tile([1, NCOMBO], BF16)
        comboag = dramp.tile([NC_, NCOMBO], BF16, addr_space="Shared")
        moe_b = dramp.tile([B, HID], BF16)
        ar_o = dramp.tile([B, HID], BF16, addr_space="Shared")

        const = top.enter_context(tc.tile_pool(name="const", bufs=1))
        ident64b = const.tile([64, 64], BF16)
        make_identity(nc, ident64b)
        ident32b = const.tile([32, 32], BF16)
        make_identity(nc, ident32b)
        ones_bf = const.tile([128, 1], BF16)
        nc.gpsimd.memset(ones_bf[:], 1.0)
        ones_row = const.tile([1, 128], F32)
        nc.gpsimd.memset(ones_row[:], 1.0)
        zero_col = const.tile([128, 1], F32)
        nc.gpsimd.memset(zero_col[:], 0.0)
        nc.const_aps.aps[(F32, 0.0)] = zero_col[:]
        epsq_col = const.tile([128, 1], F32)
        nc.gpsimd.memset(epsq_col[:], float(HID) * EPS * kq * kq)
        nc.const_aps.aps[(F32, float(HID) * EPS * kq * kq)] = epsq_col[:]
        eps_col = const.tile([128, 1], F32)
        nc.gpsimd.memset(eps_col[:], float(HID) * EPS)
        nc.const_aps.aps[(F32, float(HID) * EPS)] = eps_col[:]
        iota_sb = const.tile([128, 32], F32)
        nc.scalar.dma_start(iota_sb[:], iota_i[:])
        sel_bc = const.tile([64, 8], F32)
        nc.scalar.dma_start(sel_bc[:], sel_i.ap().to_broadcast((64, 8)))
        sel64_sb = const.tile([B, 8], BF16)
        nc.scalar.dma_start(sel64_sb[:], sel64_i[:])

        small = top.enter_context(tc.tile_pool(name="small", bufs=1))
        acts = top.enter_context(tc.tile_pool(name="acts", bufs=1))
        qT = acts.tile([128, 33, NB], BF16)        # per-slot q^T (+ kcurT at 32)
        ctxA = acts.tile([128, 64, 32], BF16)
        hidTf = acts.tile([128, 32, 64], BF16)
        xT = acts.tile([128, 32, 64], BF16)
        midT = acts.tile([128, 11 * 64], BF16)
        hsl_f = acts.tile([64, 512], F32)
        vcur1 = acts.tile([1, NB, 129], BF16)      # scaled vcur rows ++ ones col
        vx_cur = acts.tile([128, 129], BF16)       # row 0 = this slot's vcur
        ppT_cur = acts.tile([128, 32], BF16)       # row 0 = this slot's pcur
        nc.gpsimd.memset(vx_cur[:], 0.0)
        nc.gpsimd.memset(ppT_cur[:], 0.0)
        wsel_col = small.tile([64, 1], F32, name="wsel_col")

        # ---------------- Phase A: norm1 -> qkv -> AG -> slot extract --------
        with ExitStack() as pa:
            sA = pa.enter_context(tc.tile_pool(name="sA", bufs=1))
            qkvps = pa.enter_context(tc.tile_pool(name="qkvps", bufs=1, space="PSUM"))
            qtps = pa.enter_context(tc.tile_pool(name="qtps", bufs=2, space="PSUM"))

            wq_sb = sA.tile([128, 32 * 512], E3)
            nc.sync.dma_start(wq_sb[:], wqkvT_i[:])
            wk_sb = sA.tile([128, 32 * 128], E3)
            nc.sync.dma_start(wk_sb[:], wkT_i[:])
            hT = sA.tile([128, 32 * 64], BF16)
            nc.scalar.dma_start(hT[:], hidT_i[:])
            h_sb = sA.tile([B, HID], BF16)
            nc.scalar.dma_start(h_sb[:], h_i[:])
            sq_scr = sA.tile([B, HID], BF16)
            ssq = small.tile([64, 1], F32, name="ssq")
            nc.scalar.activation(sq_scr[:], h_sb[:], AF.Square, accum_out=ssq[:])
            rs_col = small.tile([64, 1], F32, name="rs_col")
            nc.scalar.activation(rs_col[:], ssq[:], AF.Sqrt,
                                 bias=float(HID) * EPS * kq * kq, scale=kq * kq)
            nc.vector.reciprocal(rs_col[:], rs_col[:])

            q1 = qkvps.tile([128, 512], F32, name="q1")
            kc = qkvps.tile([128, 128], F32, name="kc")
            for k in range(32):
                par = k % 2
                tp = (0, 64 * par)
                hk = hT[:, k * 64:(k + 1) * 64]
                nc.tensor.matmul(q1[64 * par:64 * par + 64, :], hk,
                                 wq_sb[:, k * 512:(k + 1) * 512],
                                 start=(k < 2), stop=(k >= 30), tile_position=tp)
                nc.tensor.matmul(kc[64 * par:64 * par + 64, :], hk,
                                 wk_sb[:, k * 128:(k + 1) * 128],
                                 start=(k < 2), stop=(k >= 30), tile_position=tp)
            qkv_hi = sA.tile([64, 512 + 128], F32)
            nc.scalar.activation(qkv_hi[:, :512], q1[64:128, :], AF.Copy)
            nc.scalar.activation(qkv_hi[:, 512:], kc[64:128, :], AF.Copy)
            qkv_f = sA.tile([64, 512 + 128], F32)
            nc.vector.tensor_tensor(qkv_f[:, :512], q1[0:64, :],
                                    qkv_hi[:, :512], op=ALU.add)
            nc.vector.tensor_tensor(qkv_f[:, 512:], kc[0:64, :],
                                    qkv_hi[:, 512:], op=ALU.add)
            qkv_sb = sA.tile([64, 512], BF16)
            nc.vector.tensor_scalar_mul(qkv_sb[:], qkv_f[:, :512], rs_col[:])
            kcur_sb = sA.tile([64, 128], BF16)
            nc.vector.tensor_scalar_mul(kcur_sb[:], qkv_f[:, 512:], rs_col[:])

            nc.gpsimd.dma_start(agq_in[:], qkv_sb[:])
            nc.gpsimd.collective_compute(
                "AllGather", ALU.bypass, replica_groups=rg,
                ins=[agq_in.opt()], outs=[agq.opt()],
            )
            qag_sb = sA.tile([64, 4096], BF16)
            nc.gpsimd.dma_start(
                qag_sb[:].rearrange("b (c r) -> b c r", c=8),
                agq.rearrange("c b r -> b c r"))
            # extract my 8 slots' q^T (and kcur^T) via one-hot matmuls
            for kk in range(33):
                lhs = (qag_sb[:, kk * 128:(kk + 1) * 128] if kk < 32
                       else kcur_sb[:])
                qt_ps = qtps.tile([128, 8], F32, name="qt_ps", tag="qt")
                for hh in range(2):
                    nc.tensor.matmul(qt_ps[64 * hh:64 * hh + 64, :],
                                     lhs[:, 64 * hh:64 * hh + 64], sel64_sb[:],
                                     start=True, stop=True,
                                     tile_position=(0, 64 * hh))
                nc.vector.tensor_copy(qT[:, kk, :], qt_ps[:])
            # vcur rows for my slots (scaled by KVV), with a ones column
            nc.gpsimd.memset(vcur1[:, :, 128:129], 1.0)
            for j in range(NB):
                vr = qtps.tile([1, 128], F32, name="vr", tag="vr")
                nc.tensor.matmul(vr[:], sel64_sb[:, j:j + 1], kcur_sb[:],
                                 start=True, stop=True)
                nc.scalar.activation(vcur1[0:1, j, 0:128], vr[:], AF.Copy,
                                     scale=KVV)

        # ------- bulk weight streams (SP queue, consumption order) -------
        w1p = top.enter_context(tc.tile_pool(name="w1p", bufs=3))
        w2p = top.enter_context(tc.tile_pool(name="w2p", bufs=2))
        mid_scope = top.enter_context(ExitStack())
        wstr = mid_scope.enter_context(tc.tile_pool(name="wstr", bufs=1))
        wo_sb = wstr.tile([128, 32 * 512], E3)
        nc.sync.dma_start(wo_sb[:], woT_i[:])
        gate_sb = wstr.tile([128, 32 * 8], BF16)
        nc.sync.dma_start(gate_sb[:], gateT_i[:])
        w1_tiles = [w1p.tile([128, 8 * TWO_MI], E3, name="w1t", tag="w1")
                    for _ in range(4)]
        w2_tiles = [w2p.tile([128, 4 * HID], E3, name="w2t", tag="w2")
                    for _ in range(3)]
        for t in range(3):
            nc.sync.dma_start(w1_tiles[t][:],
                              w1T_i[:, t * 8 * TWO_MI:(t + 1) * 8 * TWO_MI])
        for t, nmk in ((0, 4), (1, 4)):
            nc.sync.dma_start(w2_tiles[t][:, :nmk * HID],
                              w2T_i[:, t * 4 * HID:t * 4 * HID + nmk * HID])
        nc.sync.dma_start(w1_tiles[3][:], w1T_i[:, 3 * 8 * TWO_MI:4 * 8 * TWO_MI])
        nc.sync.dma_start(w2_tiles[2][:, :3 * HID],
                          w2T_i[:, 8 * HID:11 * HID])

        # ---------------- Phase B: attention (2 rounds x 4 slots) ------------
        with ExitStack() as pb:
            kkp = pb.enter_context(tc.tile_pool(name="kkp", bufs=12))
            kvp = pb.enter_context(tc.tile_pool(name="kvp", bufs=10))
            ppp = pb.enter_context(tc.tile_pool(name="ppp", bufs=2))
            ptp = pb.enter_context(tc.tile_pool(name="ptp", bufs=5))
            smb = pb.enter_context(tc.tile_pool(name="smb", bufs=4))
            scps = pb.enter_context(tc.tile_pool(name="scps", bufs=2, space="PSUM"))
            ctxps = pb.enter_context(tc.tile_pool(name="ctxps", bufs=2, space="PSUM"))
            ptB = pb.enter_context(tc.tile_pool(name="ptB", bufs=2, space="PSUM"))
            dps = pb.enter_context(tc.tile_pool(name="dps", bufs=2, space="PSUM"))

            sv_cols = []
            for j in range(NB):
                sv_col = smb.tile([128, 1], F32, name="sv_col", tag="sv", bufs=8)
                nc.scalar.dma_start(
                    sv_col[:], seqm1_i[0:1, j:j + 1].to_broadcast((128, 1)))
                sv_cols.append(sv_col)

            for r in range(2):
                slots = [4 * r + j4 for j4 in range(4)]
                rtcs = [tcs[j] for j in slots]
                nblk = (max(rtcs) * 128 + 1023) // 1024

                pp = ppp.tile([128, 4096], BF16, name="pp", tag="pp")
                # scores: 4 slots concurrent in PE col groups; KV pieces of
                # 1024, matmuls/exp in 512-col sub-blocks (PSUM bank limit)
                vpieces = {}
                for blk in range(nblk):
                    kpieces = {}
                    for j4, j in enumerate(slots):
                        w = min(1024, rtcs[j4] * 128 - blk * 1024)
                        if w <= 0:
                            continue
                        kt = kkp.tile([128, 1024], E3, name="kt", tag="kt")
                        nc.scalar.dma_start(kt[:, :w],
                                            kT_i[j][:, blk * 1024:blk * 1024 + w])
                        kpieces[j4] = kt
                    for sub in range(2):
                        base = blk * 1024 + sub * 512
                        if all(rtcs[j4] * 128 <= base for j4 in range(4)):
                            continue
                        sc = scps.tile([128, 512], F32, name="sc", tag="sc")
                        for j4, j in enumerate(slots):
                            w = min(512, rtcs[j4] * 128 - base)
                            if w <= 0:
                                continue
                            nc.tensor.matmul(
                                sc[32 * j4:32 * j4 + 32, :w],
                                qT[:, 0:32, j],
                                kpieces[j4][:, sub * 512:sub * 512 + w],
                                start=True, stop=True,
                                tile_position=(0, 32 * j4))
                        for j4 in range(4):
                            w = min(512, rtcs[j4] * 128 - base)
                            if w <= 0:
                                continue
                            nc.scalar.activation(
                                pp[32 * j4:32 * j4 + 32, base:base + w],
                                sc[32 * j4:32 * j4 + 32, :w], AF.Exp,
                                scale=exp_scale / KKV)
                ctx = ctxps.tile([128, 129], F32, name="ctx", tag="ctx")
                for j4, j in enumerate(slots):
                    tcj = rtcs[j4]
                    for b8 in range((tcj + 7) // 8):
                        wv = min(8 * 129, tcj * 129 - b8 * 8 * 129)
                        vt = kvp.tile([128, 8 * 129], E3, name="vt", tag="vt")
                        nc.scalar.dma_start(
                            vt[:, :wv], vx_i[j][:, b8 * 8 * 129:b8 * 8 * 129 + wv])
                        vpieces[(j4, b8)] = vt
                    # block-transpose probs into [s, (c, h)] layout, then mask
                    ppT = ptp.tile([128, 32, 32], BF16, name="ppT", tag="ppT")
                    for g in range(4):
                        nc.vector.transpose(
                            ppT[32 * g:32 * g + 32, :tcj, :],
                            pp[32 * j4:32 * j4 + 32, :]
                            .rearrange("h (c g d) -> h c (g d)", g=4, d=32)
                            [:, :tcj, 32 * g:32 * g + 32])
                    m_all = smb.tile([128, 32], BF16, name="m_all", tag="mall")
                    nc.vector.tensor_scalar(m_all[:], iota_sb[:], sv_cols[j][:],
                                            None, op0=ALU.is_lt)
                    nc.vector.tensor_tensor(
                        ppT[:, :tcj, :], ppT[:, :tcj, :],
                        m_all[:, :tcj, None].to_broadcast((128, tcj, 32)),
                        op=ALU.mult)
                    # current-token prob
                    cur = dps.tile([1, 32], F32, name="cur", tag="cur")
                    nc.tensor.matmul(cur[:], qT[:, 32, j:j + 1], qT[:, 0:32, j],
                                     start=True, stop=True)
                    nc.scalar.activation(ppT_cur[0:1, :], cur[:], AF.Exp,
                                         scale=exp_scale)
                    nc.vector.tensor_copy(vx_cur[0:1, :], vcur1[0:1, j, :])
                    # ctx (+ den in col 128 via the ones column of vx)
                    for c in range(tcj):
                        nc.tensor.matmul(
                            ctx[32 * j4:32 * j4 + 32, :],
                            ppT[:, c, :],
                            vpieces[(j4, c // 8)][:, (c % 8) * 129:(c % 8) * 129 + 129],
                            start=(c == 0), stop=False,
                            tile_position=(0, 32 * j4))
                    nc.tensor.matmul(ctx[32 * j4:32 * j4 + 32, :],
                                     ppT_cur[:], vx_cur[:],
                                     start=False, stop=True,
                                     tile_position=(0, 32 * j4))
                    den = smb.tile([32, 1], F32, name="den", tag="den")
                    nc.scalar.activation(den[:], ctx[32 * j4:32 * j4 + 32, 128:129],
                                         AF.Copy)
                    rden = smb.tile([32, 1], F32, name="rden", tag="rden")
                    nc.vector.reciprocal(rden[:], den[:])
                    ctn = smb.tile([32, 128], BF16, name="ctn", tag="ctn")
                    nc.vector.tensor_scalar_mul(
                        ctn[:], ctx[32 * j4:32 * j4 + 32, 0:128], rden[:])
                    ctnT = ptB.tile([128, 32], BF16, name="ctnT", tag="ctnT")
                    nc.tensor.transpose(ctnT[:], ctn[:], ident32b[:])
                    ctnT_sb = smb.tile([128, 32], BF16, name="ctnT_sb", tag="cts")
                    nc.vector.tensor_copy(ctnT_sb[:], ctnT[:])
                    nc.scalar.dma_start(ctx_b[:, j, :], ctnT_sb[:])

        # ------------- Phase C: AG ctx -> Wo -> hidden slice -------------
        with ExitStack() as pc:
            sC = pc.enter_context(tc.tile_pool(name="sC", bufs=1))
            wops = pc.enter_context(tc.tile_pool(name="wops", bufs=1, space="PSUM"))
            ptC = pc.enter_context(tc.tile_pool(name="ptC", bufs=2, space="PSUM"))

            nc.gpsimd.collective_compute(
                "AllGather", ALU.bypass, replica_groups=rg,
                ins=[ctx_b.opt()], outs=[agc.opt()],
            )
            nc.gpsimd.dma_start(
                ctxA[:].rearrange("d (c j) h -> d c j h", c=8),
                agc.rearrange("c d j h -> d c j h"))

            wo_ps = wops.tile([128, 512], F32)
            ctxA4 = ctxA[:].rearrange("d (c j) h -> d c j h", c=8)
            for k in range(32):
                par = k % 2
                nc.tensor.matmul(wo_ps[64 * par:64 * par + 64, :],
                                 ctxA4[:, :, :, k],
                                 wo_sb[:, k * 512:(k + 1) * 512],
                                 start=(k < 2), stop=(k >= 30),
                                 tile_position=(0, 64 * par))
            wo_hi = sC.tile([64, 512], F32)
            nc.scalar.activation(wo_hi[:], wo_ps[64:128, :], AF.Copy)
            wo_f = sC.tile([64, 512], F32)
            nc.vector.tensor_tensor(wo_f[:], wo_ps[0:64, :], wo_hi[:], op=ALU.add)
            wo_s = sC.tile([64, 512], F32)
            nc.scalar.activation(wo_s[:], wo_f[:], AF.Copy, scale=1.0 / kwo)
            hidc = sC.tile([64, 512], F32)
            nc.scalar.dma_start(hidc[:], hidc_i[:])
            nc.vector.tensor_tensor(hsl_f[:], wo_s[:], hidc[:], op=ALU.add)
            # combined AG payload (bf16): hsl ++ hslT ++ rms partial sums
            hsl_bf = sC.tile([64, 512], BF16)
            nc.vector.tensor_copy(hsl_bf[:], hsl_f[:])
            hslT = sC.tile([128, 4 * 64], BF16)
            for kk in range(4):
                pt = ptC.tile([128, 64], BF16, name="ptC_t", tag="ptC_t")
                nc.tensor.transpose(pt[:], hsl_bf[:, kk * 128:(kk + 1) * 128],
                                    ident64b[:])
                nc.vector.tensor_copy(hslT[:, kk * 64:(kk + 1) * 64], pt[:])
            sq_c = sC.tile([64, 512], BF16)
            ssqp = sC.tile([64, 1], F32)
            nc.scalar.activation(sq_c[:], hsl_bf[:], AF.Square, accum_out=ssqp[:])
            ssqp_bf = sC.tile([64, 1], BF16)
            nc.vector.tensor_copy(ssqp_bf[:], ssqp[:])
            nc.scalar.dma_start(
                combo[0, :32768].rearrange("(b o) -> b o", b=64), hsl_bf[:])
            nc.scalar.dma_start(
                combo[0, 32768:65536].rearrange("(p f) -> p f", p=128), hslT[:])
            nc.scalar.dma_start(
                combo[0, 65536:65600].rearrange("(b o) -> b o", o=1), ssqp_bf[:])
            nc.gpsimd.collective_compute(
                "AllGather", ALU.bypass, replica_groups=rg,
                ins=[combo.opt()], outs=[comboag.opt()],
            )

        # ------------- Phase D: norm2 -> x^T -> gate -> top2 -------------
        with ExitStack() as pd:
            sD = pd.enter_context(tc.tile_pool(name="sD", bufs=1))
            ssps = pd.enter_context(tc.tile_pool(name="ssps", bufs=1, space="PSUM"))
            gps = pd.enter_context(tc.tile_pool(name="gps", bufs=1, space="PSUM"))

            nc.gpsimd.dma_start(
                hidTf[:].rearrange("p (c k) b -> p c k b", c=8),
                comboag[:, 32768:65536]
                .rearrange("c (p k b) -> p c k b", p=128, k=4))
            ssqpT = sD.tile([8, 64], BF16)
            nc.gpsimd.dma_start(ssqpT[:], comboag[:, 65536:65600])
            ssq2 = ssps.tile([1, 64], F32)
            nc.tensor.matmul(ssq2[:], ones_bf[0:8, :], ssqpT[:],
                             start=True, stop=True)
            rs2 = sD.tile([1, 64], F32)
            nc.scalar.activation(rs2[:], ssq2[:], AF.Sqrt, bias=float(HID) * EPS)
            nc.vector.reciprocal(rs2[:], rs2[:])
            bc2 = ssps.tile([128, 64], F32)
            nc.tensor.matmul(bc2[:], ones_row[:], rs2[:], start=True, stop=True)
            nc.vector.tensor_tensor(
                xT[:], hidTf[:],
                bc2[:, None, :].to_broadcast((128, 32, 64)), op=ALU.mult)

            g_ps = gps.tile([64, 8], F32)
            for k in range(32):
                nc.tensor.matmul(g_ps[:], xT[:, k], gate_sb[:, k * 8:(k + 1) * 8],
                                 start=(k == 0), stop=(k == 31))
            pg = sD.tile([64, 8], F32)
            nc.scalar.activation(pg[:], g_ps[:], AF.Exp)
            m1c = sD.tile([64, 1], F32)
            nc.vector.reduce_max(m1c[:], pg[:], axis=mybir.AxisListType.X)
            eq1 = sD.tile([64, 8], F32)
            nc.vector.tensor_scalar(eq1[:], pg[:], m1c[:], None, op0=ALU.is_ge)
            t1 = sD.tile([64, 8], F32)
            nc.vector.tensor_tensor(t1[:], pg[:], eq1[:], op=ALU.mult)
            nc.vector.tensor_tensor(t1[:], pg[:], t1[:], op=ALU.subtract)
            m2c = sD.tile([64, 1], F32)
            nc.vector.reduce_max(m2c[:], t1[:], axis=mybir.AxisListType.X)
            keep = sD.tile([64, 8], F32)
            nc.vector.tensor_scalar(keep[:], pg[:], m2c[:], None, op0=ALU.is_ge)
            wsum = sD.tile([64, 1], F32)
            nc.vector.tensor_tensor(wsum[:], m1c[:], m2c[:], op=ALU.add)
            nc.vector.reciprocal(wsum[:], wsum[:])
            wts = sD.tile([64, 8], F32)
            nc.vector.tensor_tensor(wts[:], pg[:], keep[:], op=ALU.mult)
            nc.vector.tensor_scalar_mul(wts[:], wts[:], wsum[:])
            nc.vector.tensor_tensor(wts[:], wts[:], sel_bc[:], op=ALU.mult)
            nc.vector.reduce_sum(wsel_col[:], wts[:], axis=mybir.AxisListType.X)

        mid_scope.close()   # frees wo/gate SBUF before the MoE peak

        # ------------- Phase E: MoE expert FFN + AllReduce -------------
        with ExitStack() as pe1:
            gups = pe1.enter_context(tc.tile_pool(name="gups", bufs=1, space="PSUM"))
            ptE = pe1.enter_context(tc.tile_pool(name="ptE", bufs=2, space="PSUM"))
            sE = pe1.enter_context(tc.tile_pool(name="sE", bufs=1))

            gu = gups.tile([128, TWO_MI], F32)
            slices = [(o * 512, min(512, TWO_MI - o * 512)) for o in range(6)]
            for k in range(32):
                par = k % 2
                w1t = w1_tiles[k // 8]
                base = (k % 8) * TWO_MI
                for (off, w) in slices:
                    nc.tensor.matmul(gu[64 * par:64 * par + 64, off:off + w],
                                     xT[:, k],
                                     w1t[:, base + off:base + off + w],
                                     start=(k < 2), stop=(k >= 30),
                                     tile_position=(0, 64 * par))
            gu_hi = sE.tile([64, TWO_MI], F32)
            nc.scalar.activation(gu_hi[:], gu[64:128, :], AF.Copy)
            gusum = sE.tile([64, TWO_MI], BF16)
            nc.vector.tensor_tensor(gusum[:], gu[0:64, :], gu_hi[:], op=ALU.add)
            sg = sE.tile([64, MI], BF16)
            nc.scalar.activation(sg[:], gusum[:, :MI], AF.Silu, scale=1.0 / k1)
            mid = sE.tile([64, MI], BF16)
            nc.vector.tensor_tensor(mid[:], sg[:], gusum[:, MI:], op=ALU.mult)
            for mk in range(11):
                pt = ptE.tile([128, 64], BF16, name="ptE_t", tag="ptE_t")
                nc.tensor.transpose(pt[:], mid[:, mk * 128:(mk + 1) * 128],
                                    ident64b[:])
                nc.vector.tensor_copy(midT[:, mk * 64:(mk + 1) * 64], pt[:])

        with ExitStack() as pe2:
            mops = pe2.enter_context(tc.tile_pool(name="mops", bufs=1, space="PSUM"))
            sF = pe2.enter_context(tc.tile_pool(name="sF", bufs=1))
            mo = mops.tile([128, HID], F32)
            for half in range(2):
                cs = slice(half * 2048, (half + 1) * 2048)
                for mk in range(11):
                    par = mk % 2
                    w2t = w2_tiles[mk // 4]
                    base = (mk % 4) * HID + half * 2048
                    for oc in range(4):
                        nc.tensor.matmul(
                            mo[64 * par:64 * par + 64,
                               half * 2048 + oc * 512:half * 2048 + (oc + 1) * 512],
                            midT[:, mk * 64:(mk + 1) * 64],
                            w2t[:, base + oc * 512:base + (oc + 1) * 512],
                            start=(mk < 2), stop=(mk >= 9),
                            tile_position=(0, 64 * par))
                mo_hi = sF.tile([64, 2048], F32, name="mo_hi", tag="moh")
                nc.scalar.activation(mo_hi[:], mo[64:128, cs], AF.Copy,
                                     scale=wsel_col[:])
                mo_lo = sF.tile([64, 2048], F32, name="mo_lo", tag="mol")
                nc.vector.tensor_scalar_mul(mo_lo[:], mo[0:64, cs], wsel_col[:])
                mo_w = sF.tile([64, 2048], BF16, name="mo_w", tag="mow")
                nc.vector.tensor_tensor(mo_w[:], mo_lo[:], mo_hi[:], op=ALU.add)
                nc.scalar.dma_start(moe_b[:, cs], mo_w[:])
            nc.gpsimd.collective_compute(
                "AllReduce", ALU.add, replica_groups=rg,
                ins=[moe_b.opt()], outs=[ar_o.opt()],
            )

            hidf = sF.tile([64, HID], BF16)
            nc.gpsimd.dma_start(
                hidf[:].rearrange("b (c o) -> b c o", c=8),
                comboag[:, :32768].rearrange("c (b o) -> b c o", b=64))
            ar_sb = sF.tile([64, HID], BF16)
            nc.gpsimd.dma_start(ar_sb[:], ar_o[:])
            out_sb = sF.tile([64, HID], F32)
            nc.vector.tensor_tensor(out_sb[:], ar_sb[:], hidf[:], op=ALU.add)
            nc.scalar.dma_start(out_o[:], out_sb[:])

    nc.compile()
    return nc


_NC_CACHE = None
_CACHE_KEY = None


def kernel(hidden_states, positions, k_cache, v_cache, seq_lens,
           norm1_w, norm2_w, Wqkv, Wo, gate_w, w1, w2):
    global LAST_RESULT, _NC_CACHE, _CACHE_KEY

    hs = np.asarray(hidden_states, np.float32).reshape(B, HID)
    seq = np.asarray(seq_lens, np.int32)
    n1 = np.asarray(norm1_w, np.float32) * 64.0
    n2 = np.asarray(norm2_w, np.float32) * 64.0

    # sort batches by seq_len desc, deal round-robin: core c slot j gets
    # original batch P[8c+j] = order[8j+c]; slot trip count from slot max.
    order = np.argsort(-seq, kind="stable")
    P = np.empty(B, np.int64)
    for j in range(NB):
        for c in range(NC_):
            P[NC_ * c + j] = order[NB * j + c]
    tcs = tuple(int(math.ceil(max(int(seq[order[NB * j]]), 1) / 128.0))
                for j in range(NB))

    hs_p = hs[P]
    wq = np.asarray(Wqkv, np.float32)[:QROWS] * n1[None, :]
    kq = _pow2_scale(wq)
    wo_fold = np.asarray(Wo, np.float32) * (1.0 / KVV)
    kwo = _pow2_scale(wo_fold)
    gT_full = (np.asarray(gate_w, np.float32) * n2[None, :])
    w1n = np.asarray(w1, np.float32) * n2[None, None, :]
    k1 = _pow2_scale(w1n)
    w2f = np.asarray(w2, np.float32)
    k2 = _pow2_scale(w2f)

    key = (tcs, kq, kwo, k1)
    if _NC_CACHE is None or _CACHE_KEY != key:
        _NC_CACHE = _build_program(tcs, kq, kwo, k1)
        _CACHE_KEY = key
    nc = _NC_CACHE

    hidT = np.ascontiguousarray(
        hs_p.T.reshape(32, 128, 64).transpose(1, 0, 2).reshape(128, 32 * 64)
    ).astype(BF)
    iota2d = (np.arange(128, dtype=np.float32)[:, None]
              + 128.0 * np.arange(32, dtype=np.float32)[None, :])
    seqm1_p = (seq[P].astype(np.float32) - 1.0)

    khat = _e3(np.asarray(k_cache, np.float32) * KKV)
    vhat = _e3(np.asarray(v_cache, np.float32) * KVV)
    wk_pack = _pack32(_e3(wq[4096:QROWS] * kq).astype(np.float32), 32, 128
                      ).astype(E3M4)
    gate_pack = _pack32(gT_full, 32, 8).astype(BF)
    ones129 = np.ones((NB, 128, 32, 1), E3M4)

    in_maps = []
    for c in range(NC_):
        Pc = P[c * NB:(c + 1) * NB]
        sel = np.zeros((1, 8), np.float32)
        sel[0, c] = 1.0 / (k1 * k2)
        sel64 = np.zeros((B, 8), BF)
        for j in range(NB):
            sel64[NB * c + j, j] = 1.0
        wq_c = _e3(wq[c * 512:(c + 1) * 512] * kq)           # (512, 4096)
        wo_c = _e3(wo_fold[c * 512:(c + 1) * 512] * kwo)     # (512, 4096)
        w1_c = _e3(w1n[c] * k1)                              # (2816, 4096)
        w2_c = _e3(w2f[c] * k2)                              # (4096, 1408)
        vv = vhat[Pc].reshape(NB, 32, 128, HD).transpose(0, 2, 1, 3)
        vx = np.concatenate([vv, ones129], axis=3).reshape(NB, 128, 32 * 129)
        in_maps.append({
            "hidT": hidT,
            "hbf": hs_p.astype(BF),
            "hidcols": np.ascontiguousarray(hs_p[:, c * 512:(c + 1) * 512]),
            "wqkvT": _pack32(wq_c.astype(np.float32), 32, 512).astype(E3M4),
            "wkT": wk_pack,
            "woT": _pack32(wo_c.astype(np.float32), 32, 512).astype(E3M4),
            "gateT": gate_pack,
            "w1T": _pack32(w1_c.astype(np.float32), 32, TWO_MI).astype(E3M4),
            "w2T": _pack32(w2_c.astype(np.float32), 11, HID).astype(E3M4),
            "kT": np.ascontiguousarray(khat[Pc].transpose(0, 2, 1)),
            "vx": np.ascontiguousarray(vx),
            "seqm1": np.ascontiguousarray(seqm1_p[c * NB:(c + 1) * NB]
                                          .reshape(1, NB)),
            "sel": sel,
            "sel64": sel64,
            "iota2d": iota2d,
        })

    LAST_RESULT = run_bass_kernel_spmd(nc, in_maps, core_ids=list(range(NC_)))
    res_p = LAST_RESULT.results[0]["out"]
    out = np.empty((B, HID), np.float32)
    out[P] = res_p
    return out.reshape(B, 1, HID).astype(np.float32)


# revision 56
# speedup vs baseline: 2.1298x; 1.0192x over previous
"""DeepSeek-V2 decode layer on 8 TRN2 NeuronCores (Bass/Tile SPMD kernel), v3.

Sharding (per core c of 8):
  - QKV proj row-parallel (512 q-rows/core, e3m4); current-token k/v rows
    (128) replicated on every core. Per-core q^T slots are extracted from an
    AllGather (bf16) of the row shards via one-hot matmuls (no AllToAll, no
    core-dependent addressing).
  - Attention data-parallel over batch: batches sorted by seq_len, dealt
    round-robin so slot j has compile-time trip count tc[j]; KV cache e3m4.
    Scores run 4 slots concurrently in PE column groups (q^T stationary,
    kT streaming, s-blocks of 1024); probs are block-transposed on the DVE,
    masked, then ctx runs 4-slot col-grouped with v+ones moving (the ones
    column yields the softmax denominator for free).
  - ctx -> AllGather (bf16) -> Wo col-parallel (e3m4) -> hidden slice fp32.
    One combined AllGather carries hidden^T (bf16, for the MoE x^T path) and
    hidden (fp32, for the exact final residual, consumed late).
  - MoE expert-parallel dense, w1/w2 e3m4, paired PE column tiling for M=64;
    final AllReduce in bf16, 2 column chunks, overlapped with w2 compute.
Dequant folds: kq^2 into rsqrt(ms), HD^-0.5/kkv into the Exp scale, kvv into
Wo host-side, 1/(k1*k2) into the expert one-hot `sel`, 1/kwo into a copy.
"""

import os
import sys
import math

import numpy as np
import ml_dtypes

for _p in ("/opt/trn_rl_repo", "/root/.axon_site/_ro/trn_rl_repo", "/root/.axon_site"):
    if _p not in sys.path and os.path.isdir(_p):
        sys.path.append(_p)


def _ensure_ntff_hook():
    """This image's antenv lacks axon_hooks; shim it so BASS_TRACE works."""
    import types

    try:
        import antenv.axon_hooks  # noqa: F401
        return
    except ImportError:
        pass
    import antenv

    mod = types.ModuleType("antenv.axon_hooks")
    _state = {"h": None}
    mod.set_axon_ntff_profile_hook = lambda h: _state.__setitem__("h", h)
    mod.get_axon_ntff_profile_hook = lambda: _state["h"]
    sys.modules["antenv.axon_hooks"] = mod
    antenv.axon_hooks = mod
    try:
        sys.path.insert(0, "/root/.axon_site/trn_agent_boot")
        import trn_boot

        so_path = "/opt/axon/libaxon_pjrt.so"
        if os.path.exists(so_path):
            mod.set_axon_ntff_profile_hook(
                trn_boot._ntff_profile_via_ctypes(so_path))
    except Exception as e:  # tracing degrades; compile+run still work
        print(f"ntff hook install failed: {e}")


_ensure_ntff_hook()

import concourse.bacc as bacc
import concourse.bass as bass
import concourse.mybir as mybir
import concourse.tile as tile
from concourse.bass_utils import run_bass_kernel_spmd
from concourse.masks import make_identity
from contextlib import ExitStack

F32 = mybir.dt.float32
BF16 = mybir.dt.bfloat16
E3 = mybir.dt.float8e3
AF = mybir.ActivationFunctionType
ALU = mybir.AluOpType

B, HID, S, NH, HD = 64, 4096, 4096, 32, 128
QROWS = NH * HD + HD          # 4224 used rows of Wqkv (q + current-k)
NB = B // 8                   # 8 batches (slots) per core
MI, TWO_MI = 1408, 2816
NC_ = 8
EPS = 1e-6
E3M4 = ml_dtypes.float8_e3m4
BF = ml_dtypes.bfloat16
KKV = 2.0                     # host scale on k cache
KVV = 2.0                     # host scale on v cache

LAST_RESULT = None            # BassKernelResults of the most recent run


def _pow2_scale(x, target=2.0):
    s = float(np.asarray(x, np.float32).std())
    if s <= 0:
        return 1.0
    return 2.0 ** round(math.log2(target / s))


def _e3(x):
    return np.clip(np.asarray(x, np.float32), -15.0, 15.0).astype(E3M4)


def _pack32(wT, nk, ncols):
    """[ncols, K=nk*128] weight (row-major) -> [128, nk*ncols] chunk-packed:
    pack[p, k*ncols + r] = wT[r, k*128 + p]."""
    return np.ascontiguousarray(
        wT.T.reshape(nk, 128, ncols).transpose(1, 0, 2).reshape(128, nk * ncols))


def _build_program(tcs, cmins, kq, kwo, k1):
    nc = bacc.Bacc(None, target_bir_lowering=False, num_devices=NC_)

    hidT_i = nc.dram_tensor("hidT", [128, 32 * 64], BF16, kind="ExternalInput")
    h_i = nc.dram_tensor("hbf", [B, HID], BF16, kind="ExternalInput")
    hidc_i = nc.dram_tensor("hidcols", [B, 512], F32, kind="ExternalInput")
    wqkvT_i = nc.dram_tensor("wqkvT", [128, 32 * 512], E3, kind="ExternalInput")
    wkT_i = nc.dram_tensor("wkT", [128, 32 * 128], E3, kind="ExternalInput")
    woT_i = nc.dram_tensor("woT", [128, 32 * 512], E3, kind="ExternalInput")
    gateT_i = nc.dram_tensor("gateT", [128, 32 * 8], BF16, kind="ExternalInput")
    w1T_i = nc.dram_tensor("w1T", [128, 32 * TWO_MI], E3, kind="ExternalInput")
    w2T_i = nc.dram_tensor("w2T", [128, 11 * HID], E3, kind="ExternalInput")
    kT_i = nc.dram_tensor("kT", [NB, HD, S], E3, kind="ExternalInput")
    vx_i = nc.dram_tensor("vx", [NB, 128, 32 * 129], E3, kind="ExternalInput")
    seqm1_i = nc.dram_tensor("seqm1", [1, NB], F32, kind="ExternalInput")
    sel_i = nc.dram_tensor("sel", [1, 8], F32, kind="ExternalInput")
    sel64_i = nc.dram_tensor("sel64", [B, 8], BF16, kind="ExternalInput")
    iota_i = nc.dram_tensor("iota2d", [128, 32], F32, kind="ExternalInput")
    out_o = nc.dram_tensor("out", [B, HID], F32, kind="ExternalOutput")

    rg = [list(range(NC_))]
    exp_scale = float(HD) ** -0.5
    # combined hidden AllGather payload (all bf16): hsl (32768) ++ hslT
    # (32768) ++ per-core rms partial sums (64)
    NCOMBO = 65600

    with tile.TileContext(nc) as tc, ExitStack() as top:
        dramp = top.enter_context(tc.tile_pool(name="dram", bufs=1, space="DRAM"))
        agq_in = dramp.tile([B, 512], BF16)
        agq = dramp.tile([NC_, B, 512], BF16, addr_space="Shared")
        ctx_b = dramp.tile([HD, NB, 32], BF16)
        agc = dramp.tile([NC_, HD, NB, 32], BF16, addr_space="Shared")
        combo = dramp.tile([1, NCOMBO], BF16)
        comboag = dramp.tile([NC_, NCOMBO], BF16, addr_space="Shared")
        moe_b = dramp.tile([B, HID], BF16)
        ar_o = dramp.tile([B, HID], BF16, addr_space="Shared")

        const = top.enter_context(tc.tile_pool(name="const", bufs=1))
        ident64b = const.tile([64, 64], BF16)
        make_identity(nc, ident64b)
        ident32b = const.tile([32, 32], BF16)
        make_identity(nc, ident32b)
        ones_bf = const.tile([128, 1], BF16)
        nc.gpsimd.memset(ones_bf[:], 1.0)
        ones_row = const.tile([1, 128], F32)
        nc.gpsimd.memset(ones_row[:], 1.0)
        zero_col = const.tile([128, 1], F32)
        nc.gpsimd.memset(zero_col[:], 0.0)
        nc.const_aps.aps[(F32, 0.0)] = zero_col[:]
        epsq_col = const.tile([128, 1], F32)
        nc.gpsimd.memset(epsq_col[:], float(HID) * EPS * kq * kq)
        nc.const_aps.aps[(F32, float(HID) * EPS * kq * kq)] = epsq_col[:]
        eps_col = const.tile([128, 1], F32)
        nc.gpsimd.memset(eps_col[:], float(HID) * EPS)
        nc.const_aps.aps[(F32, float(HID) * EPS)] = eps_col[:]
        iota_sb = const.tile([128, 32], F32)
        nc.scalar.dma_start(iota_sb[:], iota_i[:])
        sel_bc = const.tile([64, 8], F32)
        nc.scalar.dma_start(sel_bc[:], sel_i.ap().to_broadcast((64, 8)))
        sel64_sb = const.tile([B, 8], BF16)
        nc.scalar.dma_start(sel64_sb[:], sel64_i[:])

        small = top.enter_context(tc.tile_pool(name="small", bufs=1))
        acts = top.enter_context(tc.tile_pool(name="acts", bufs=1))
        qT = acts.tile([128, 33, NB], BF16)        # per-slot q^T (+ kcurT at 32)
        ctxA = acts.tile([128, 64, 32], BF16)
        hidTf = acts.tile([128, 32, 64], BF16)
        xT = acts.tile([128, 32, 64], BF16)
        midT = acts.tile([128, 11 * 64], BF16)
        hsl_f = acts.tile([64, 512], F32)
        vcur1 = acts.tile([1, NB, 129], BF16)      # scaled vcur rows ++ ones col
        vx_cur = acts.tile([128, 129], BF16)       # row 0 = this slot's vcur
        ppT_cur = acts.tile([128, 32], BF16)       # row 0 = this slot's pcur
        nc.gpsimd.memset(vx_cur[:], 0.0)
        nc.gpsimd.memset(ppT_cur[:], 0.0)
        wsel_col = small.tile([64, 1], F32, name="wsel_col")

        # ---------------- Phase A: norm1 -> qkv -> AG -> slot extract --------
        with ExitStack() as pa:
            sA = pa.enter_context(tc.tile_pool(name="sA", bufs=1))
            qkvps = pa.enter_context(tc.tile_pool(name="qkvps", bufs=1, space="PSUM"))
            qtps = pa.enter_context(tc.tile_pool(name="qtps", bufs=2, space="PSUM"))

            wq_sb = sA.tile([128, 32 * 512], E3)
            nc.sync.dma_start(wq_sb[:], wqkvT_i[:])
            wk_sb = sA.tile([128, 32 * 128], E3)
            nc.sync.dma_start(wk_sb[:], wkT_i[:])
            hT = sA.tile([128, 32 * 64], BF16)
            nc.scalar.dma_start(hT[:], hidT_i[:])
            h_sb = sA.tile([B, HID], BF16)
            nc.scalar.dma_start(h_sb[:], h_i[:])
            sq_scr = sA.tile([B, HID], BF16)
            ssq = small.tile([64, 1], F32, name="ssq")
            nc.scalar.activation(sq_scr[:], h_sb[:], AF.Square, accum_out=ssq[:])
            rs_col = small.tile([64, 1], F32, name="rs_col")
            nc.scalar.activation(rs_col[:], ssq[:], AF.Sqrt,
                                 bias=float(HID) * EPS * kq * kq, scale=kq * kq)
            nc.vector.reciprocal(rs_col[:], rs_col[:])

            q1 = qkvps.tile([128, 512], F32, name="q1")
            kc = qkvps.tile([128, 128], F32, name="kc")
            for k in range(32):
                par = k % 2
                tp = (0, 64 * par)
                hk = hT[:, k * 64:(k + 1) * 64]
                nc.tensor.matmul(q1[64 * par:64 * par + 64, :], hk,
                                 wq_sb[:, k * 512:(k + 1) * 512],
                                 start=(k < 2), stop=(k >= 30), tile_position=tp)
                nc.tensor.matmul(kc[64 * par:64 * par + 64, :], hk,
                                 wk_sb[:, k * 128:(k + 1) * 128],
                                 start=(k < 2), stop=(k >= 30), tile_position=tp)
            qkv_hi = sA.tile([64, 512 + 128], F32)
            nc.scalar.activation(qkv_hi[:, :512], q1[64:128, :], AF.Copy)
            nc.scalar.activation(qkv_hi[:, 512:], kc[64:128, :], AF.Copy)
            qkv_f = sA.tile([64, 512 + 128], F32)
            nc.vector.tensor_tensor(qkv_f[:, :512], q1[0:64, :],
                                    qkv_hi[:, :512], op=ALU.add)
            nc.vector.tensor_tensor(qkv_f[:, 512:], kc[0:64, :],
                                    qkv_hi[:, 512:], op=ALU.add)
            qkv_sb = sA.tile([64, 512], BF16)
            nc.vector.tensor_scalar_mul(qkv_sb[:], qkv_f[:, :512], rs_col[:])
            kcur_sb = sA.tile([64, 128], BF16)
            nc.vector.tensor_scalar_mul(kcur_sb[:], qkv_f[:, 512:], rs_col[:])

            nc.gpsimd.dma_start(agq_in[:], qkv_sb[:])
            nc.gpsimd.collective_compute(
                "AllGather", ALU.bypass, replica_groups=rg,
                ins=[agq_in.opt()], outs=[agq.opt()],
            )
            qag_sb = sA.tile([64, 4096], BF16)
            nc.gpsimd.dma_start(
                qag_sb[:].rearrange("b (c r) -> b c r", c=8),
                agq.rearrange("c b r -> b c r"))
            # extract my 8 slots' q^T (and kcur^T) via one-hot matmuls
            for kk in range(33):
                lhs = (qag_sb[:, kk * 128:(kk + 1) * 128] if kk < 32
                       else kcur_sb[:])
                qt_ps = qtps.tile([128, 8], F32, name="qt_ps", tag="qt")
                for hh in range(2):
                    nc.tensor.matmul(qt_ps[64 * hh:64 * hh + 64, :],
                                     lhs[:, 64 * hh:64 * hh + 64], sel64_sb[:],
                                     start=True, stop=True,
                                     tile_position=(0, 64 * hh))
                nc.vector.tensor_copy(qT[:, kk, :], qt_ps[:])
            # vcur rows for my slots (scaled by KVV), with a ones column
            nc.gpsimd.memset(vcur1[:, :, 128:129], 1.0)
            for j in range(NB):
                vr = qtps.tile([1, 128], F32, name="vr", tag="vr")
                nc.tensor.matmul(vr[:], sel64_sb[:, j:j + 1], kcur_sb[:],
                                 start=True, stop=True)
                nc.scalar.activation(vcur1[0:1, j, 0:128], vr[:], AF.Copy,
                                     scale=KVV)

        # ------- bulk weight streams (SP queue, consumption order) -------
        w1p = top.enter_context(tc.tile_pool(name="w1p", bufs=6))
        w2p = top.enter_context(tc.tile_pool(name="w2p", bufs=5))
        mid_scope = top.enter_context(ExitStack())
        wstr = mid_scope.enter_context(tc.tile_pool(name="wstr", bufs=1))
        wo_sb = wstr.tile([128, 32 * 512], E3)
        nc.sync.dma_start(wo_sb[:], woT_i[:])
        gate_sb = wstr.tile([128, 32 * 8], BF16)
        nc.sync.dma_start(gate_sb[:], gateT_i[:])
        w1_tiles = [w1p.tile([128, 4 * TWO_MI], E3, name="w1t", tag="w1")
                    for _ in range(8)]
        w2_tiles = [w2p.tile([128, 2 * HID], E3, name="w2t", tag="w2")
                    for _ in range(6)]
        for t in range(8):
            nc.sync.dma_start(w1_tiles[t][:],
                              w1T_i[:, t * 4 * TWO_MI:(t + 1) * 4 * TWO_MI])
        for t in range(6):
            nmk = 2 if t < 5 else 1
            nc.sync.dma_start(w2_tiles[t][:, :nmk * HID],
                              w2T_i[:, t * 2 * HID:t * 2 * HID + nmk * HID])

        # ---------------- Phase B: attention (2 rounds x 4 slots) ------------
        with ExitStack() as pb:
            kvp = pb.enter_context(tc.tile_pool(name="kvp", bufs=5))
            ppp = pb.enter_context(tc.tile_pool(name="ppp", bufs=1))
            ptp = pb.enter_context(tc.tile_pool(name="ptp", bufs=4))
            smb = pb.enter_context(tc.tile_pool(name="smb", bufs=4))
            scps = pb.enter_context(tc.tile_pool(name="scps", bufs=2, space="PSUM"))
            ctxps = pb.enter_context(tc.tile_pool(name="ctxps", bufs=2, space="PSUM"))
            ptB = pb.enter_context(tc.tile_pool(name="ptB", bufs=2, space="PSUM"))
            dps = pb.enter_context(tc.tile_pool(name="dps", bufs=2, space="PSUM"))

            sv_cols = []
            for j in range(NB):
                sv_col = smb.tile([128, 1], F32, name="sv_col", tag="sv", bufs=8)
                nc.scalar.dma_start(
                    sv_col[:], seqm1_i[0:1, j:j + 1].to_broadcast((128, 1)))
                sv_cols.append(sv_col)

            for r in range(2):
                slots = [4 * r + j4 for j4 in range(4)]
                rtcs = [tcs[j] for j in slots]
                nblk = (max(rtcs) * 128 + 1023) // 1024

                pp = ppp.tile([128, 4096], BF16, name="pp", tag="pp")
                # full-slot KV loads (big DMAs), round-robin across slots
                kts, vxs = {}, {}
                for j4, j in enumerate(slots):
                    tcj = rtcs[j4]
                    kt = kvp.tile([128, S], E3, name="kt", tag="kt")
                    nc.scalar.dma_start(kt[:, :tcj * 128],
                                        kT_i[j][:, :tcj * 128])
                    vt = kvp.tile([128, 32 * 129], E3, name="vt", tag="vt")
                    nc.scalar.dma_start(vt[:, :tcj * 129],
                                        vx_i[j][:, :tcj * 129])
                    kts[j4], vxs[j4] = kt, vt
                # scores: 4 slots concurrent in PE col groups, 512-col blocks
                for base in range(0, max(rtcs) * 128, 512):
                    sc = scps.tile([128, 512], F32, name="sc", tag="sc")
                    for j4, j in enumerate(slots):
                        w = min(512, rtcs[j4] * 128 - base)
                        if w <= 0:
                            continue
                        nc.tensor.matmul(
                            sc[32 * j4:32 * j4 + 32, :w],
                            qT[:, 0:32, j],
                            kts[j4][:, base:base + w],
                            start=True, stop=True,
                            tile_position=(0, 32 * j4))
                    for j4 in range(4):
                        w = min(512, rtcs[j4] * 128 - base)
                        if w <= 0:
                            continue
                        nc.scalar.activation(
                            pp[32 * j4:32 * j4 + 32, base:base + w],
                            sc[32 * j4:32 * j4 + 32, :w], AF.Exp,
                            scale=exp_scale / KKV)
                ctx = ctxps.tile([128, 129], F32, name="ctx", tag="ctx")
                for j4, j in enumerate(slots):
                    tcj = rtcs[j4]
                    cm = cmins[j]
                    # block-transpose probs into [s, (c, h)] layout, then mask
                    # (only chunks >= cmin can contain invalid positions)
                    ppT = ptp.tile([128, 32, 32], BF16, name="ppT", tag="ppT")
                    for g in range(4):
                        nc.vector.transpose(
                            ppT[32 * g:32 * g + 32, :tcj, :],
                            pp[32 * j4:32 * j4 + 32, :]
                            .rearrange("h (c g d) -> h c (g d)", g=4, d=32)
                            [:, :tcj, 32 * g:32 * g + 32])
                    m_all = smb.tile([128, 32], BF16, name="m_all", tag="mall")
                    nc.vector.tensor_scalar(m_all[:], iota_sb[:], sv_cols[j][:],
                                            None, op0=ALU.is_lt)
                    nc.vector.tensor_tensor(
                        ppT[:, cm:tcj, :], ppT[:, cm:tcj, :],
                        m_all[:, cm:tcj, None].to_broadcast((128, tcj - cm, 32)),
                        op=ALU.mult)
                    # current-token prob
                    cur = dps.tile([1, 32], F32, name="cur", tag="cur")
                    nc.tensor.matmul(cur[:], qT[:, 32, j:j + 1], qT[:, 0:32, j],
                                     start=True, stop=True)
                    nc.scalar.activation(ppT_cur[0:1, :], cur[:], AF.Exp,
                                         scale=exp_scale)
                    nc.vector.tensor_copy(vx_cur[0:1, :], vcur1[0:1, j, :])
                    # ctx (+ den in col 128 via the ones column of vx)
                    for c in range(tcj):
                        nc.tensor.matmul(
                            ctx[32 * j4:32 * j4 + 32, :],
                            ppT[:, c, :],
                            vxs[j4][:, c * 129:(c + 1) * 129],
                            start=(c == 0), stop=False,
                            tile_position=(0, 32 * j4))
                    nc.tensor.matmul(ctx[32 * j4:32 * j4 + 32, :],
                                     ppT_cur[:], vx_cur[:],
                                     start=False, stop=True,
                                     tile_position=(0, 32 * j4))
                    den = smb.tile([32, 1], F32, name="den", tag="den")
                    nc.scalar.activation(den[:], ctx[32 * j4:32 * j4 + 32, 128:129],
                                         AF.Copy)
                    rden = smb.tile([32, 1], F32, name="rden", tag="rden")
                    nc.vector.reciprocal(rden[:], den[:])
                    ctn = smb.tile([32, 128], BF16, name="ctn", tag="ctn")
                    nc.vector.tensor_scalar_mul(
                        ctn[:], ctx[32 * j4:32 * j4 + 32, 0:128], rden[:])
                    ctnT = ptB.tile([128, 32], BF16, name="ctnT", tag="ctnT")
                    nc.tensor.transpose(ctnT[:], ctn[:], ident32b[:])
                    ctnT_sb = smb.tile([128, 32], BF16, name="ctnT_sb", tag="cts")
                    nc.vector.tensor_copy(ctnT_sb[:], ctnT[:])
                    nc.scalar.dma_start(ctx_b[:, j, :], ctnT_sb[:])

        # ------------- Phase C: AG ctx -> Wo -> hidden slice -------------
        with ExitStack() as pc:
            sC = pc.enter_context(tc.tile_pool(name="sC", bufs=1))
            wops = pc.enter_context(tc.tile_pool(name="wops", bufs=1, space="PSUM"))
            ptC = pc.enter_context(tc.tile_pool(name="ptC", bufs=2, space="PSUM"))

            nc.gpsimd.collective_compute(
                "AllGather", ALU.bypass, replica_groups=rg,
                ins=[ctx_b.opt()], outs=[agc.opt()],
            )
            nc.gpsimd.dma_start(
                ctxA[:].rearrange("d (c j) h -> d c j h", c=8),
                agc.rearrange("c d j h -> d c j h"))

            wo_ps = wops.tile([128, 512], F32)
            ctxA4 = ctxA[:].rearrange("d (c j) h -> d c j h", c=8)
            for k in range(32):
                par = k % 2
                nc.tensor.matmul(wo_ps[64 * par:64 * par + 64, :],
                                 ctxA4[:, :, :, k],
                                 wo_sb[:, k * 512:(k + 1) * 512],
                                 start=(k < 2), stop=(k >= 30),
                                 tile_position=(0, 64 * par))
            wo_hi = sC.tile([64, 512], F32)
            nc.scalar.activation(wo_hi[:], wo_ps[64:128, :], AF.Copy)
            wo_f = sC.tile([64, 512], F32)
            nc.vector.tensor_tensor(wo_f[:], wo_ps[0:64, :], wo_hi[:], op=ALU.add)
            wo_s = sC.tile([64, 512], F32)
            nc.scalar.activation(wo_s[:], wo_f[:], AF.Copy, scale=1.0 / kwo)
            hidc = sC.tile([64, 512], F32)
            nc.scalar.dma_start(hidc[:], hidc_i[:])
            nc.vector.tensor_tensor(hsl_f[:], wo_s[:], hidc[:], op=ALU.add)
            # combined AG payload (bf16): hsl ++ hslT ++ rms partial sums
            hsl_bf = sC.tile([64, 512], BF16)
            nc.vector.tensor_copy(hsl_bf[:], hsl_f[:])
            hslT = sC.tile([128, 4 * 64], BF16)
            for kk in range(4):
                pt = ptC.tile([128, 64], BF16, name="ptC_t", tag="ptC_t")
                nc.tensor.transpose(pt[:], hsl_bf[:, kk * 128:(kk + 1) * 128],
                                    ident64b[:])
                nc.vector.tensor_copy(hslT[:, kk * 64:(kk + 1) * 64], pt[:])
            sq_c = sC.tile([64, 512], BF16)
            ssqp = sC.tile([64, 1], F32)
            nc.scalar.activation(sq_c[:], hsl_bf[:], AF.Square, accum_out=ssqp[:])
            ssqp_bf = sC.tile([64, 1], BF16)
            nc.vector.tensor_copy(ssqp_bf[:], ssqp[:])
            nc.scalar.dma_start(
                combo[0, :32768].rearrange("(b o) -> b o", b=64), hsl_bf[:])
            nc.scalar.dma_start(
                combo[0, 32768:65536].rearrange("(p f) -> p f", p=128), hslT[:])
            nc.scalar.dma_start(
                combo[0, 65536:65600].rearrange("(b o) -> b o", o=1), ssqp_bf[:])
            nc.gpsimd.collective_compute(
                "AllGather", ALU.bypass, replica_groups=rg,
                ins=[combo.opt()], outs=[comboag.opt()],
            )

        # ------------- Phase D: norm2 -> x^T -> gate -> top2 -------------
        with ExitStack() as pd:
            sD = pd.enter_context(tc.tile_pool(name="sD", bufs=1))
            ssps = pd.enter_context(tc.tile_pool(name="ssps", bufs=1, space="PSUM"))
            gps = pd.enter_context(tc.tile_pool(name="gps", bufs=1, space="PSUM"))

            nc.gpsimd.dma_start(
                hidTf[:].rearrange("p (c k) b -> p c k b", c=8),
                comboag[:, 32768:65536]
                .rearrange("c (p k b) -> p c k b", p=128, k=4))
            ssqpT = sD.tile([8, 64], BF16)
            nc.gpsimd.dma_start(ssqpT[:], comboag[:, 65536:65600])
            ssq2 = ssps.tile([1, 64], F32)
            nc.tensor.matmul(ssq2[:], ones_bf[0:8, :], ssqpT[:],
                             start=True, stop=True)
            rs2 = sD.tile([1, 64], F32)
            nc.scalar.activation(rs2[:], ssq2[:], AF.Sqrt, bias=float(HID) * EPS)
            nc.vector.reciprocal(rs2[:], rs2[:])
            bc2 = ssps.tile([128, 64], F32)
            nc.tensor.matmul(bc2[:], ones_row[:], rs2[:], start=True, stop=True)
            nc.vector.tensor_tensor(
                xT[:], hidTf[:],
                bc2[:, None, :].to_broadcast((128, 32, 64)), op=ALU.mult)

            g_ps = gps.tile([64, 8], F32)
            for k in range(32):
                nc.tensor.matmul(g_ps[:], xT[:, k], gate_sb[:, k * 8:(k + 1) * 8],
                                 start=(k == 0), stop=(k == 31))
            pg = sD.tile([64, 8], F32)
            nc.scalar.activation(pg[:], g_ps[:], AF.Exp)
            m1c = sD.tile([64, 1], F32)
            nc.vector.reduce_max(m1c[:], pg[:], axis=mybir.AxisListType.X)
            eq1 = sD.tile([64, 8], F32)
            nc.vector.tensor_scalar(eq1[:], pg[:], m1c[:], None, op0=ALU.is_ge)
            t1 = sD.tile([64, 8], F32)
            nc.vector.tensor_tensor(t1[:], pg[:], eq1[:], op=ALU.mult)
            nc.vector.tensor_tensor(t1[:], pg[:], t1[:], op=ALU.subtract)
            m2c = sD.tile([64, 1], F32)
            nc.vector.reduce_max(m2c[:], t1[:], axis=mybir.AxisListType.X)
            keep = sD.tile([64, 8], F32)
            nc.vector.tensor_scalar(keep[:], pg[:], m2c[:], None, op0=ALU.is_ge)
            wsum = sD.tile([64, 1], F32)
            nc.vector.tensor_tensor(wsum[:], m1c[:], m2c[:], op=ALU.add)
            nc.vector.reciprocal(wsum[:], wsum[:])
            wts = sD.tile([64, 8], F32)
            nc.vector.tensor_tensor(wts[:], pg[:], keep[:], op=ALU.mult)
            nc.vector.tensor_scalar_mul(wts[:], wts[:], wsum[:])
            nc.vector.tensor_tensor(wts[:], wts[:], sel_bc[:], op=ALU.mult)
            nc.vector.reduce_sum(wsel_col[:], wts[:], axis=mybir.AxisListType.X)

        mid_scope.close()   # frees wo/gate SBUF before the MoE peak

        # ------------- Phase E: MoE expert FFN + AllReduce -------------
        with ExitStack() as pe1:
            gups = pe1.enter_context(tc.tile_pool(name="gups", bufs=1, space="PSUM"))
            ptE = pe1.enter_context(tc.tile_pool(name="ptE", bufs=2, space="PSUM"))
            sE = pe1.enter_context(tc.tile_pool(name="sE", bufs=1))

            gu = gups.tile([128, TWO_MI], F32)
            slices = [(o * 512, min(512, TWO_MI - o * 512)) for o in range(6)]
            for k in range(32):
                par = k % 2
                w1t = w1_tiles[k // 4]
                base = (k % 4) * TWO_MI
                for (off, w) in slices:
                    nc.tensor.matmul(gu[64 * par:64 * par + 64, off:off + w],
                                     xT[:, k],
                                     w1t[:, base + off:base + off + w],
                                     start=(k < 2), stop=(k >= 30),
                                     tile_position=(0, 64 * par))
            gu_hi = sE.tile([64, TWO_MI], F32)
            nc.scalar.activation(gu_hi[:], gu[64:128, :], AF.Copy)
            gusum = sE.tile([64, TWO_MI], BF16)
            nc.vector.tensor_tensor(gusum[:], gu[0:64, :], gu_hi[:], op=ALU.add)
            sg = sE.tile([64, MI], BF16)
            nc.scalar.activation(sg[:], gusum[:, :MI], AF.Silu, scale=1.0 / k1)
            mid = sE.tile([64, MI], BF16)
            nc.vector.tensor_tensor(mid[:], sg[:], gusum[:, MI:], op=ALU.mult)
            for mk in range(11):
                pt = ptE.tile([128, 64], BF16, name="ptE_t", tag="ptE_t")
                nc.tensor.transpose(pt[:], mid[:, mk * 128:(mk + 1) * 128],
                                    ident64b[:])
                nc.vector.tensor_copy(midT[:, mk * 64:(mk + 1) * 64], pt[:])

        with ExitStack() as pe2:
            mops = pe2.enter_context(tc.tile_pool(name="mops", bufs=1, space="PSUM"))
            sF = pe2.enter_context(tc.tile_pool(name="sF", bufs=1))
            mo = mops.tile([128, HID], F32)
            for half in range(2):
                cs = slice(half * 2048, (half + 1) * 2048)
                for mk in range(11):
                    par = mk % 2
                    w2t = w2_tiles[mk // 2]
                    base = (mk % 2) * HID + half * 2048
                    for oc in range(4):
                        nc.tensor.matmul(
                            mo[64 * par:64 * par + 64,
                               half * 2048 + oc * 512:half * 2048 + (oc + 1) * 512],
                            midT[:, mk * 64:(mk + 1) * 64],
                            w2t[:, base + oc * 512:base + (oc + 1) * 512],
                            start=(mk < 2), stop=(mk >= 9),
                            tile_position=(0, 64 * par))
                mo_hi = sF.tile([64, 2048], F32, name="mo_hi", tag="moh")
                nc.scalar.activation(mo_hi[:], mo[64:128, cs], AF.Copy,
                                     scale=wsel_col[:])
                mo_lo = sF.tile([64, 2048], F32, name="mo_lo", tag="mol")
                nc.vector.tensor_scalar_mul(mo_lo[:], mo[0:64, cs], wsel_col[:])
                mo_w = sF.tile([64, 2048], BF16, name="mo_w", tag="mow")
                nc.vector.tensor_tensor(mo_w[:], mo_lo[:], mo_hi[:], op=ALU.add)
                nc.scalar.dma_start(moe_b[:, cs], mo_w[:])
            hidf = sF.tile([64, HID], BF16)
            nc.gpsimd.dma_start(
                hidf[:].rearrange("b (c o) -> b c o", c=8),
                comboag[:, :32768].rearrange("c (b o) -> b c o", b=64))
            nc.gpsimd.collective_compute(
                "AllReduce", ALU.add, replica_groups=rg,
                ins=[moe_b.opt()], outs=[ar_o.opt()],
            )

            ar_sb = sF.tile([64, HID], BF16)
            nc.gpsimd.dma_start(ar_sb[:], ar_o[:])
            out_sb = sF.tile([64, HID], F32)
            nc.vector.tensor_tensor(out_sb[:], ar_sb[:], hidf[:], op=ALU.add)
            nc.scalar.dma_start(out_o[:], out_sb[:])

    nc.compile()
    return nc


_NC_CACHE = None
_CACHE_KEY = None


def kernel(hidden_states, positions, k_cache, v_cache, seq_lens,
           norm1_w, norm2_w, Wqkv, Wo, gate_w, w1, w2):
    global LAST_RESULT, _NC_CACHE, _CACHE_KEY

    hs = np.asarray(hidden_states, np.float32).reshape(B, HID)
    seq = np.asarray(seq_lens, np.int32)
    n1 = np.asarray(norm1_w, np.float32) * 64.0
    n2 = np.asarray(norm2_w, np.float32) * 64.0

    # sort batches by seq_len desc, deal round-robin: core c slot j gets
    # original batch P[8c+j] = order[8j+c]; slot trip count from slot max.
    order = np.argsort(-seq, kind="stable")
    P = np.empty(B, np.int64)
    for j in range(NB):
        for c in range(NC_):
            P[NC_ * c + j] = order[NB * j + c]
    tcs = tuple(int(math.ceil(max(int(seq[order[NB * j]]), 1) / 128.0))
                for j in range(NB))
    cmins = tuple(max(0, (int(seq[order[NB * j + NC_ - 1]]) - 1) // 128)
                  for j in range(NB))

    hs_p = hs[P]
    wq = np.asarray(Wqkv, np.float32)[:QROWS] * n1[None, :]
    kq = _pow2_scale(wq)
    wo_fold = np.asarray(Wo, np.float32) * (1.0 / KVV)
    kwo = _pow2_scale(wo_fold)
    gT_full = (np.asarray(gate_w, np.float32) * n2[None, :])
    w1n = np.asarray(w1, np.float32) * n2[None, None, :]
    k1 = _pow2_scale(w1n)
    w2f = np.asarray(w2, np.float32)
    k2 = _pow2_scale(w2f)

    key = (tcs, cmins, kq, kwo, k1)
    if _NC_CACHE is None or _CACHE_KEY != key:
        _NC_CACHE = _build_program(tcs, cmins, kq, kwo, k1)
        _CACHE_KEY = key
    nc = _NC_CACHE

    hidT = np.ascontiguousarray(
        hs_p.T.reshape(32, 128, 64).transpose(1, 0, 2).reshape(128, 32 * 64)
    ).astype(BF)
    iota2d = (np.arange(128, dtype=np.float32)[:, None]
              + 128.0 * np.arange(32, dtype=np.float32)[None, :])
    seqm1_p = (seq[P].astype(np.float32) - 1.0)

    khat = _e3(np.asarray(k_cache, np.float32) * KKV)
    vhat = _e3(np.asarray(v_cache, np.float32) * KVV)
    wk_pack = _pack32(_e3(wq[4096:QROWS] * kq).astype(np.float32), 32, 128
                      ).astype(E3M4)
    gate_pack = _pack32(gT_full, 32, 8).astype(BF)
    ones129 = np.ones((NB, 128, 32, 1), E3M4)

    in_maps = []
    for c in range(NC_):
        Pc = P[c * NB:(c + 1) * NB]
        sel = np.zeros((1, 8), np.float32)
        sel[0, c] = 1.0 / (k1 * k2)
        sel64 = np.zeros((B, 8), BF)
        for j in range(NB):
            sel64[NB * c + j, j] = 1.0
        wq_c = _e3(wq[c * 512:(c + 1) * 512] * kq)           # (512, 4096)
        wo_c = _e3(wo_fold[c * 512:(c + 1) * 512] * kwo)     # (512, 4096)
        w1_c = _e3(w1n[c] * k1)                              # (2816, 4096)
        w2_c = _e3(w2f[c] * k2)                              # (4096, 1408)
        vv = vhat[Pc].reshape(NB, 32, 128, HD).transpose(0, 2, 1, 3)
        vx = np.concatenate([vv, ones129], axis=3).reshape(NB, 128, 32 * 129)
        in_maps.append({
            "hidT": hidT,
            "hbf": hs_p.astype(BF),
            "hidcols": np.ascontiguousarray(hs_p[:, c * 512:(c + 1) * 512]),
            "wqkvT": _pack32(wq_c.astype(np.float32), 32, 512).astype(E3M4),
            "wkT": wk_pack,
            "woT": _pack32(wo_c.astype(np.float32), 32, 512).astype(E3M4),
            "gateT": gate_pack,
            "w1T": _pack32(w1_c.astype(np.float32), 32, TWO_MI).astype(E3M4),
            "w2T": _pack32(w2_c.astype(np.float32), 11, HID).astype(E3M4),
            "kT": np.ascontiguousarray(khat[Pc].transpose(0, 2, 1)),
            "vx": np.ascontiguousarray(vx),
            "seqm1": np.ascontiguousarray(seqm1_p[c * NB:(c + 1) * NB]
                                          .reshape(1, NB)),
            "sel": sel,
            "sel64": sel64,
            "iota2d": iota2d,
        })

    LAST_RESULT = run_bass_kernel_spmd(nc, in_maps, core_ids=list(range(NC_)))
    res_p = LAST_RESULT.results[0]["out"]
    out = np.empty((B, HID), np.float32)
    out[P] = res_p
    return out.reshape(B, 1, HID).astype(np.float32)
